# revision 1
# baseline (speedup 1.0000x reference)
"""Stereo cost-volume + softmax disparity regression + bilinear upsample.

Full inputs:  feat_l, feat_r [16, 4, 128, 240] f32, img_h=1024, img_w=1920.
Full output:  [16, 1, 1024, 1920] f32.

Sharding: pure data parallel, 2 samples per core across 8 cores.

Per-core layout: SBUF partitions p = ch*32 + (y % 32); free dim packs
(y_block, x) = 4*240 = 960 columns per sample; the two samples are
processed as a software pipeline (sample 1's cost volume overlaps sample
0's upsample) so DVE-heavy phase 1 and copy-heavy phase 2 share the span.

Per sample:
  1. cost volume: fp16 DVE subtract against a left-zero-padded feat_r
     (two pad copies so even/odd disparity shifts stay 4B-aligned for the
     DVE 2x mode), abs in place (even d: DVE bitwise-and sign clear at 4x;
     odd d: ACT Abs), channel-sum via PE selector matmuls (col-tiled,
     4 disparities per pass) into 1-bank PSUM chunks.
  2. softmax regression: ACT exp(8-cost) -> fp16, PE matmul with
     [ones ; 8*d] weights accumulating (s, t) in PSUM f32 over all 24
     disparities, DVE reciprocal + multiply -> pred fp16.
  3. upsample (align_corners bilinear = two dense fp16 matmuls): DMA xbar
     transpose pred -> predT, M1: tmp[y, X] = predT.T @ WxT (X padded to
     2048 so all PSUM chunks are 512-col bank-aligned), M2: out[Y, X] =
     WyT_chunk.T @ tmp, PSUM -> SBUF copies alternating DVE/ACT, fp16 DMA
     to HBM, host casts to f32.

All matmul PSUM outputs are <= 512 f32 columns and bank-aligned (a 480-col
chunk at offset 480 would straddle the 2 KiB bank boundary and silently
corrupt). PSUM budget: cost/tmp shared 1-bank slots (2 banks) + s/t
accumulators (4) + out chunks (2) = 8 banks, so both pipeline stages hold
their PSUM concurrently.
"""

import sys

sys.path.insert(0, "/opt/trn_rl_repo")

import numpy as np

import concourse.bacc as bacc
import concourse.tile as tile
import concourse.mybir as mybir
from concourse.bass_utils import run_bass_kernel_spmd

# ---------------------------------------------------------------- constants
B, C, H0, W0 = 16, 4, 128, 240
D = 24             # disparities
NCORES = 8
SPC = B // NCORES  # samples per core = 2
HI, WI = 1024, 1920
WP = WI            # X chunked as 512,512,512,384 (bank-aligned starts)
XCH = [(0, 512), (512, 512), (1024, 512), (1536, 384)]
YB = H0 // 32      # 4 y-blocks
G = SPC * YB       # 8 feat groups (sample-major)
FREE = G * W0      # 1920
PAD = 28           # left-pad columns in padded feat_r groups (>= D+2, even)
GW = W0 + 2 * PAD  # padded group width (even)
EXP_BIAS = 8.0

FP16 = mybir.dt.float16
F32 = mybir.dt.float32
U16 = mybir.dt.uint16

_TRACE = [False]


# ------------------------------------------------------------- host weights
def _host_consts():
    # selector for channel sum: sel[ch*32+y32, m] = (y32 == m)
    sel = np.zeros((128, 32), np.float16)
    for ch in range(C):
        sel[ch * 32 : (ch + 1) * 32, :] = np.eye(32, dtype=np.float16)

    # s/t weights per disparity group g: stw[dj*32+y32, m]
    #   m in [0,32): s-selector (ones);  m in [32,64): t = 8*d selector
    stw = np.zeros((128, 6 * 64), np.float16)
    for g in range(6):
        for dj in range(4):
            d = 4 * g + dj
            blk = stw[dj * 32 : (dj + 1) * 32, g * 64 : (g + 1) * 64]
            blk[:, 0:32] = np.eye(32, dtype=np.float16)
            blk[:, 32:64] = np.eye(32, dtype=np.float16) * np.float16(8.0 * d)

    # x-interp weights wxT[x, X], f32 linspace to match jnp rounding
    xs = np.linspace(0.0, W0 - 1.0, WI, dtype=np.float32)
    x0 = np.floor(xs).astype(np.int64)
    x1 = np.minimum(x0 + 1, W0 - 1)
    wx = (xs - x0).astype(np.float32)
    wxT_full = np.zeros((W0, WI), np.float32)
    wxT_full[x0, np.arange(WI)] += 1.0 - wx
    wxT_full[x1, np.arange(WI)] += wx
    # two overlapping 128-row x-chunks (DMA transpose needs 128-col blocks);
    # the 16 overlap rows (x 112:128) are zeroed in chunk B
    wxT = np.zeros((256, WI), np.float32)
    wxT[0:128] = wxT_full[0:128]
    wxT[144:256] = wxT_full[128:240]

    # y-interp weights wyT[y, Y]
    ys = np.linspace(0.0, H0 - 1.0, HI, dtype=np.float32)
    y0 = np.floor(ys).astype(np.int64)
    y1 = np.minimum(y0 + 1, H0 - 1)
    wy = (ys - y0).astype(np.float32)
    wyT = np.zeros((H0, HI), np.float32)
    wyT[y0, np.arange(HI)] += 1.0 - wy
    wyT[y1, np.arange(HI)] += wy

    ident = np.eye(128, dtype=np.float16)
    return {
        "sel": sel,
        "stw": stw,
        "wxT": wxT.astype(np.float16),
        "wyT": wyT.astype(np.float16),
        "ident": ident,
    }


def _pack_feat(f):
    """[SPC, C, H0, W0] -> [128, FREE] with p=(ch,y32), free=(s,yb,x)."""
    a = f.reshape(SPC, C, YB, 32, W0)
    a = np.ascontiguousarray(a.transpose(1, 3, 0, 2, 4))  # ch,y32,s,yb,x
    return a.reshape(128, FREE)


# ------------------------------------------------------------- build kernel
def _build():
    nc = bacc.Bacc("TRN2", target_bir_lowering=False, debug=False,
                   num_devices=NCORES)
    lf = nc.dram_tensor("lf", [128, FREE], FP16, kind="ExternalInput").ap()
    rf = nc.dram_tensor("rf", [128, FREE], FP16, kind="ExternalInput").ap()
    sel_d = nc.dram_tensor("sel", [128, 32], FP16, kind="ExternalInput").ap()
    stw_d = nc.dram_tensor("stw", [128, 384], FP16, kind="ExternalInput").ap()
    wxT_d = nc.dram_tensor("wxT", [256, WI], FP16, kind="ExternalInput").ap()
    wyT_d = nc.dram_tensor("wyT", [H0, HI], FP16, kind="ExternalInput").ap()
    idn_d = nc.dram_tensor("ident", [128, 128], FP16,
                           kind="ExternalInput").ap()
    out = nc.dram_tensor("out", [SPC, HI, WI], FP16,
                         kind="ExternalOutput").ap()

    AF = mybir.ActivationFunctionType
    OP = mybir.AluOpType

    with tile.TileContext(nc) as tc:
        with (
            tc.tile_pool(name="consts", bufs=1) as consts,
            tc.tile_pool(name="feat", bufs=1) as feat,
            tc.tile_pool(name="diff", bufs=8) as diffp,
            tc.tile_pool(name="ep", bufs=6) as ep,
            tc.tile_pool(name="predp", bufs=1) as predp,
            tc.tile_pool(name="upsb", bufs=1) as upsb,
            tc.tile_pool(name="outsb", bufs=6) as outsb,
            # PSUM budget (8 banks): ps1 (cost/tmp share 1-bank slots) = 2,
            # st (2 samples x [64,1024] f32) = 4, out chunks = 2
            tc.tile_pool(name="ps1", bufs=2, space="PSUM") as ps1,
            tc.tile_pool(name="outps", bufs=2, space="PSUM") as outps,
        ):
            from contextlib import ExitStack
            st_stack = ExitStack()
            stps = st_stack.enter_context(
                tc.tile_pool(name="stps", bufs=1, space="PSUM"))
            # ---- features first (phase-1 critical path), parallel queues
            L = feat.tile([128, FREE], FP16)
            nc.sync.dma_start(out=L, in_=lf)
            L3 = L.rearrange("p (g w) -> p g w", w=W0)
            # padded feat_r, two copies for even/odd shift alignment
            rf3 = rf.rearrange("p (g w) -> p g w", w=W0)
            R = []  # R[par][h] -> [128, YB, GW] view
            for par in range(2):
                Rh = []
                for h2 in range(SPC):
                    Rt = feat.tile([128, YB * GW], FP16,
                                   tag=f"rpad{par}{h2}",
                                   name=f"rpad{par}{h2}")
                    nc.gpsimd.memset(Rt, 0.0)
                    Rv = Rt.rearrange("p (g w) -> p g w", w=GW)
                    dma_eng = nc.scalar if par == 0 else nc.gpsimd
                    dma_eng.dma_start(
                        out=Rv[:, :, PAD + par : PAD + par + W0],
                        in_=rf3[:, YB * h2 : YB * h2 + YB, :],
                    )
                    Rh.append(Rv)
                R.append(Rh)

            # ---- constants (needed a bit later) on the scalar queue
            sel = consts.tile([128, 32], FP16)
            nc.gpsimd.dma_start(out=sel, in_=sel_d)
            stw = consts.tile([128, 384], FP16)
            nc.gpsimd.dma_start(out=stw, in_=stw_d)
            wxT = [consts.tile([128, WI], FP16, name=f"wxT{i}", tag=f"wxT{i}")
                   for i in range(2)]
            nc.gpsimd.dma_start(out=wxT[0], in_=wxT_d[0:128, :])
            nc.gpsimd.dma_start(out=wxT[1], in_=wxT_d[128:256, :])
            wyT = consts.tile([128, HI], FP16)
            nc.gpsimd.dma_start(out=wyT, in_=wyT_d)
            idn = consts.tile([128, 128], FP16)
            nc.gpsimd.dma_start(out=idn, in_=idn_d)
            bias8 = consts.tile([128, 1], F32)
            nc.vector.memset(bias8, EXP_BIAS)

            st = [stps.tile([64, 1024], F32, name=f"st{h}", tag=f"st{h}")
                  for h in range(SPC)]

            copy_tick = [0]

            def psum_copy(dst, src, dve_mod=2):
                if copy_tick[0] % dve_mod == 0:
                    nc.vector.tensor_copy(out=dst, in_=src)
                else:
                    nc.scalar.copy(out=dst, in_=src)
                copy_tick[0] += 1

            # ============ software pipeline over the two samples =========
            # Engine instruction streams are in-order, so sample 0's
            # upsample work is interleaved with sample 1's cost volume at
            # emission time to avoid head-of-line blocking.
            pred = [None, None]

            def emit_ph1_group(h, g):
                gs = slice(YB * h, YB * h + YB)
                absd = []
                for dj in range(4):
                    d = 4 * g + dj
                    par = d % 2
                    off = PAD + par - d
                    diff = diffp.tile([128, YB, W0], FP16, name="diff",
                                      tag="diff")
                    nc.vector.tensor_tensor(
                        out=diff, in0=L3[:, gs, :],
                        in1=R[par][h][:, :, off : off + W0],
                        op=OP.subtract,
                    )
                    # sample 0 runs alone (ACT idle): split abs by parity;
                    # sample 1 overlaps sample 0's copies: abs all on DVE
                    if h == 0 and d % 2 == 1:
                        nc.scalar.activation(out=diff, in_=diff, func=AF.Abs)
                    else:
                        di = diff.bitcast(U16)
                        nc.vector.tensor_scalar(
                            out=di, in0=di, scalar1=0x7FFF, scalar2=None,
                            op0=OP.bitwise_and,
                        )
                    absd.append(diff.rearrange("p g w -> p (g w)"))
                e = ep.tile([128, 1024], FP16, name="e", tag="e")
                for nch in range(2):
                    cost = ps1.tile([128, 512], F32, name="cost", tag="ps1")
                    for dj in range(4):
                        nc.tensor.matmul(
                            cost[dj * 32 : dj * 32 + 32, 0:480],
                            lhsT=sel,
                            rhs=absd[dj][:, nch * 480 : nch * 480 + 480],
                            start=True, stop=True,
                            tile_position=(0, dj * 32),
                        )
                    nc.scalar.activation(
                        out=e[:, nch * 512 : nch * 512 + 480],
                        in_=cost[:, 0:480], func=AF.Exp,
                        bias=bias8, scale=-1.0)
                for nch in range(2):
                    nc.tensor.matmul(
                        st[h][0:64, nch * 512 : nch * 512 + 480],
                        lhsT=stw[:, g * 64 : g * 64 + 64],
                        rhs=e[:, nch * 512 : nch * 512 + 480],
                        start=(g == 0), stop=(g == 5),
                        tile_position=(0, 0),
                        skip_group_check=True,
                    )

            def emit_pred(h):
                rs = predp.tile([32, 1024], F32, name=f"rs{h}", tag=f"rs{h}")
                pr = predp.tile([32, 1024], FP16, name=f"pred{h}",
                                tag=f"pred{h}")
                for nch in range(2):
                    sl = slice(nch * 512, nch * 512 + 480)
                    nc.vector.reciprocal(out=rs[:, sl], in_=st[h][0:32, sl])
                    nc.vector.tensor_tensor(out=pr[:, sl],
                                            in0=st[h][32:64, sl],
                                            in1=rs[:, sl], op=OP.mult)
                pred[h] = pr

            def emit_ph2_head(h, dve_mod, pool=None):
                """transposes + M1 -> tmp_sb for sample h"""
                pr = pred[h]
                predT = []
                for xh in range(2):
                    pt_ps = (pool() if pool else
                             ps1.tile([128, 512], F32, name="pt_ps",
                                      tag="ps1")).bitcast(FP16)
                    for yb in range(YB):
                        pcol = (yb // 2) * 512 + (yb % 2) * W0
                        nc.tensor.transpose(
                            pt_ps[0:128, yb * 32 : yb * 32 + 32],
                            pr[0:32, pcol + xh * 112 :
                               pcol + xh * 112 + 128],
                            idn[0:32, 0:32],
                        )
                    pt = upsb.tile([128, 128], FP16, tag=f"predT{h}{xh}",
                                   name=f"predT{h}{xh}")
                    nc.scalar.copy(out=pt, in_=pt_ps[0:128, 0:128])
                    predT.append(pt)
                tmp_sb = upsb.tile([128, WP], FP16, tag=f"tmp{h}",
                                   name=f"tmp{h}")
                for c0, nw in XCH:
                    t_ps = (pool() if pool else
                            ps1.tile([128, 512], F32, name="t_ps",
                                     tag="ps1"))
                    for xh in range(2):
                        nc.tensor.matmul(
                            t_ps[:, 0:nw], lhsT=predT[xh],
                            rhs=wxT[xh][:, c0 : c0 + nw],
                            start=(xh == 0), stop=(xh == 1),
                        )
                    psum_copy(tmp_sb[:, c0 : c0 + nw], t_ps[:, 0:nw],
                              dve_mod)
                return tmp_sb

            dma_tick = [0]

            def emit_ph2_yc(h, tmp_sb, yc, dve_mod, pool=None):
                ob = outsb.tile([128, WP], FP16, name="ob", tag="ob")
                for c0, nw in XCH:
                    o_ps = (pool() if pool else
                            outps.tile([128, 512], F32, name="o_ps",
                                       tag="o_ps"))
                    nc.tensor.matmul(
                        o_ps[:, 0:nw],
                        lhsT=wyT[:, yc * 128 : yc * 128 + 128],
                        rhs=tmp_sb[:, c0 : c0 + nw],
                        start=True, stop=True,
                    )
                    psum_copy(ob[:, c0 : c0 + nw], o_ps[:, 0:nw], dve_mod)
                eng = nc.sync if dma_tick[0] % 2 == 0 else nc.gpsimd
                dma_tick[0] += 1
                eng.dma_start(
                    out=out[h, yc * 128 : yc * 128 + 128, :],
                    in_=ob)

            # sample 0 cost volume + regression
            for g in range(6):
                emit_ph1_group(0, g)
            emit_pred(0)
            # interleave: sample 1 phase 1 with sample 0 upsample.
            # s1's first groups go first so DVE has runway while s0's
            # transposes/M1 chain resolves; head copies stay off DVE.
            emit_ph1_group(1, 0)
            tmp0 = emit_ph2_head(0, dve_mod=10**9)
            s0_yc = 0
            for g in range(1, 6):
                emit_ph1_group(1, g)
                if s0_yc < 5:
                    emit_ph2_yc(0, tmp0, s0_yc, dve_mod=5)
                    s0_yc += 1
            emit_pred(1)
            # finish sample 0 on the existing pools (no WAR on st banks)
            while s0_yc < 8:
                emit_ph2_yc(0, tmp0, s0_yc, dve_mod=2)
                s0_yc += 1
            tmp1 = emit_ph2_head(1, dve_mod=2)
            st_stack.close()  # free the 4 s/t banks for the tail
            with tc.tile_pool(name="pstail", bufs=4, space="PSUM") as pstail:
                tailps = [pstail]

                def tail_tile():
                    return tailps[0].tile([128, 512], F32, name="tl",
                                          tag="tl")

                for yc in range(8):
                    emit_ph2_yc(1, tmp1, yc, dve_mod=2, pool=tail_tile)
    nc.compile()
    return nc


_NC_CACHE = [None]


def kernel(feat_l, feat_r, img_h, img_w):
    feat_l = np.asarray(feat_l, dtype=np.float32)
    feat_r = np.asarray(feat_r, dtype=np.float32)
    assert int(img_h) == HI and int(img_w) == WI
    assert feat_l.shape == (B, C, H0, W0)

    if _NC_CACHE[0] is None:
        _NC_CACHE[0] = _build()
    nc = _NC_CACHE[0]

    consts = _host_consts()
    in_maps = []
    for c in range(NCORES):
        fl = _pack_feat(feat_l[SPC * c : SPC * c + SPC].astype(np.float16))
        fr = _pack_feat(feat_r[SPC * c : SPC * c + SPC].astype(np.float16))
        in_maps.append({"lf": fl, "rf": fr, **consts})

    res = run_bass_kernel_spmd(nc, in_maps, core_ids=list(range(NCORES)),
                               trace=_TRACE[0])
    outs = [res.results[i]["out"].astype(np.float32) for i in range(NCORES)]
    full = np.concatenate(outs, axis=0).reshape(B, 1, HI, WI)
    kernel._last_exec_ns = res.exec_time_ns
    return full



# revision 64
# speedup vs baseline: 1.2377x; 1.2377x over previous
"""Stereo cost-volume + softmax disparity regression + bilinear upsample.

Full inputs:  feat_l, feat_r [16, 4, 128, 240] f32, img_h=1024, img_w=1920.
Full output:  [16, 1, 1024, 1920] f32.

Sharding: pure data parallel, 2 samples per core across 8 cores.

Layout: SBUF partitions p = ch*32 + (y % 32); free dim packs (y_block, x).
The two samples run as a software pipeline (sample 1's cost volume
overlaps sample 0's upsample).

Phase 1 (per disparity group g of 4):
  - DVE (or Pool for a few groups) computes |L - R(x-d)| for all 4
    disparities in ONE subtract (custom 4D access pattern walking the
    padded feat_r window) + ONE 4x-mode bitwise abs.
  - Channel sum runs "flipped" on the PE: the diff chunk [128, 120] is
    the stationary lhsT and the [128, 32] selector streams, producing
    cost chunks [120(x), 32(y32)] -- 4x fewer streamed columns than
    streaming the diff.  Output layout: cost[x-block, (sec, yb, xb, y32)].
  - ACT exponentiates a whole group tile [120, 1024] at once.
  - s/t accumulate in PSUM via scaled-identity lhsT matmuls (s += e,
    t += 8d*e), one [120, 32] region per (sec, yb, xb); PSUM lazy-zero
    semantics allow interleaved region accumulation with start exactly
    once per bank.
Phase 2: pred = t * (1/s) comes out ALREADY x-transposed [120(x), (yb,
  xb, y32)], so M1 (x-interp) consumes it directly as lhsT -- no PE
  transposes.  M1 splits X at the exact pure-A/pure-B boundary columns
  (955/964) so only an 8-column sliver needs both x-halves.  M2
  (y-interp) streams tmp through wyT chunks.  PSUM->SBUF copies are
  spread over ACT/Pool (and DVE in the tail); output rows DMA on the
  otherwise idle SP queue plus Pool in the tail.
"""

import sys

sys.path.insert(0, "/opt/trn_rl_repo")

import numpy as np

import concourse.bass as bass
import concourse.bacc as bacc
import concourse.tile as tile
import concourse.mybir as mybir
from concourse.bass_utils import run_bass_kernel_spmd

# ---------------------------------------------------------------- constants
B, C, H0, W0 = 16, 4, 128, 240
D = 24             # disparities
NCORES = 8
SPC = B // NCORES  # samples per core = 2
HI, WI = 1024, 1920
WP = WI
XB = 120           # x-block width (two blocks per row)
XSPLIT = 956       # X column where the x-interp flips from wxT half A to B
# M2 / output X chunks (PSUM <= 512 cols each, split at XSPLIT)
XCH = [(0, 512), (512, 444), (956, 512), (1468, 452)]
# M1 X chunks: (start, width, x-halves needed); grouped so chunks 0-1 and
# 2-4 each pack into one 2-bank PSUM tile without bank-straddling writes
XCH_M1 = [(0, 512, (0,)), (512, 444, (0,)), (956, 8, (0, 1)),
          (964, 504, (1,)), (1468, 452, (1,))]
YB = H0 // 32      # 4 y-blocks
G = SPC * YB       # 8 feat groups (sample-major)
FREE = G * W0      # 1920
PAD = 28           # left-pad columns in padded feat_r groups
GW = W0 + 2 * PAD  # padded group width (even)
EXP_BIAS = 8.0

FP16 = mybir.dt.float16
F32 = mybir.dt.float32
U16 = mybir.dt.uint16

_TRACE = [False]


# ------------------------------------------------------------- host weights
def _host_consts():
    # selector for the flipped channel sum: sel[ch*32+y32, y'] = (y32 == y')
    sel = np.zeros((128, 32), np.float16)
    for ch in range(C):
        sel[ch * 32 : (ch + 1) * 32, :] = np.eye(32, dtype=np.float16)

    # s identity and per-disparity t identities (8*d scaling)
    sid = np.eye(XB, dtype=np.float16)
    tid = np.zeros((XB, D * XB), np.float16)
    for d in range(D):
        tid[:, d * XB : (d + 1) * XB] = np.eye(XB, dtype=np.float16) * \
            np.float16(8.0 * d)

    # x-interp weights wxT[x, X], f32 linspace to match jnp rounding
    xs = np.linspace(0.0, W0 - 1.0, WI, dtype=np.float32)
    x0 = np.floor(xs).astype(np.int64)
    x1 = np.minimum(x0 + 1, W0 - 1)
    wx = (xs - x0).astype(np.float32)
    wxT_full = np.zeros((W0, WI), np.float32)
    wxT_full[x0, np.arange(WI)] += 1.0 - wx
    wxT_full[x1, np.arange(WI)] += wx
    # chunk validity: columns left of 956 only use x<120; right of 964 only
    # x>=120; the 8-col sliver uses both
    assert x1[:956].max() <= XB - 1
    assert x0[964:].min() >= XB
    wxTa = wxT_full[0:XB]
    wxTb = wxT_full[XB : 2 * XB]

    # y-interp weights wyT[y, Y]
    ys = np.linspace(0.0, H0 - 1.0, HI, dtype=np.float32)
    y0 = np.floor(ys).astype(np.int64)
    y1 = np.minimum(y0 + 1, H0 - 1)
    wy = (ys - y0).astype(np.float32)
    wyT = np.zeros((H0, HI), np.float32)
    wyT[y0, np.arange(HI)] += 1.0 - wy
    wyT[y1, np.arange(HI)] += wy

    return {
        "sel": sel,
        "sid": sid,
        "tid": tid,
        "wxTa": wxTa.astype(np.float16),
        "wxTb": wxTb.astype(np.float16),
        "wyT": wyT.astype(np.float16),
    }


def _pack_feat(f):
    """[SPC, C, H0, W0] -> [128, FREE] with p=(ch,y32), free=(s,yb,x)."""
    a = f.reshape(SPC, C, YB, 32, W0)
    a = np.ascontiguousarray(a.transpose(1, 3, 0, 2, 4))  # ch,y32,s,yb,x
    return a.reshape(128, FREE)


def _pack_feat_padded(f):
    """[SPC, C, H0, W0] -> [128, SPC*YB*GW], PAD zero cols around each row."""
    a = f.reshape(SPC, C, YB, 32, W0).transpose(1, 3, 0, 2, 4)
    p = np.zeros((C, 32, SPC, YB, GW), f.dtype)
    p[:, :, :, :, PAD : PAD + W0] = a
    return p.reshape(128, SPC * YB * GW)


# scheduling configuration (engine assignment knobs, tuned via sweep)
CFG = {
    "pool_g0": (1, 3),
    "pool_g1": (1, 3),
    "mid_pat": "AAV",
    "tail_pat": "AV",
    "dma_pat": "SPSA",    # tail out-DMA queues: S=SP, A=ACT, P=Pool
    "mult1": "V",         # engine for sample-1 pred multiply
}


# ------------------------------------------------------------- build kernel
def _build(cfg=None):
    cfg = {**CFG, **(cfg or {})}
    nc = bacc.Bacc("TRN2", target_bir_lowering=False, debug=False,
                   num_devices=NCORES)
    lf = nc.dram_tensor("lf", [128, FREE], FP16, kind="ExternalInput").ap()
    rf = nc.dram_tensor("rf", [128, SPC * YB * GW], FP16,
                        kind="ExternalInput").ap()
    sel_d = nc.dram_tensor("sel", [128, 32], FP16, kind="ExternalInput").ap()
    sid_d = nc.dram_tensor("sid", [XB, XB], FP16, kind="ExternalInput").ap()
    tid_d = nc.dram_tensor("tid", [XB, D * XB], FP16,
                           kind="ExternalInput").ap()
    wxa_d = nc.dram_tensor("wxTa", [XB, WI], FP16, kind="ExternalInput").ap()
    wxb_d = nc.dram_tensor("wxTb", [XB, WI], FP16, kind="ExternalInput").ap()
    wyT_d = nc.dram_tensor("wyT", [H0, HI], FP16, kind="ExternalInput").ap()
    out = nc.dram_tensor("out", [SPC, HI, WI], FP16,
                         kind="ExternalOutput").ap()

    AF = mybir.ActivationFunctionType
    OP = mybir.AluOpType

    with tile.TileContext(nc) as tc:
        with (
            tc.tile_pool(name="consts", bufs=1) as consts,
            tc.tile_pool(name="feat", bufs=1) as feat,
            tc.tile_pool(name="diff", bufs=4) as diffp,
            tc.tile_pool(name="ep", bufs=4) as ep,
            tc.tile_pool(name="predp", bufs=1) as predp,
            tc.tile_pool(name="upsb", bufs=1) as upsb,
            tc.tile_pool(name="outsb", bufs=6) as outsb,
            tc.tile_pool(name="outps", bufs=2, space="PSUM") as outps,
        ):
            from contextlib import ExitStack
            ph1_stack = ExitStack()
            costp = ph1_stack.enter_context(
                tc.tile_pool(name="costp", bufs=2, space="PSUM"))
            stps = ph1_stack.enter_context(
                tc.tile_pool(name="stps", bufs=1, space="PSUM"))

            bias8 = consts.tile([XB, 1], F32)
            nc.vector.memset(bias8, EXP_BIAS)

            # ---- features: left halves on SP; padded right h0 on Pool
            # (ahead of ACT's table load), h1 on ACT.
            # per-sample L tiles so sample-0's subs don't wait on h1's DMA
            Lh = []
            for h2 in range(SPC):
                Lt = feat.tile([128, FREE // 2], FP16, tag=f"L{h2}",
                               name=f"L{h2}")
                nc.sync.dma_start(
                    out=Lt,
                    in_=lf[:, h2 * (FREE // 2) : (h2 + 1) * (FREE // 2)])
                Lh.append(Lt.rearrange("p (g w) -> p g w", w=W0))
            R = [None, None]  # R[h] -> [128, YB, GW]
            for h2 in range(SPC):
                Rt = feat.tile([128, YB * GW], FP16,
                               tag=f"rpad{h2}", name=f"rpad{h2}")
                dma_eng = nc.gpsimd if h2 == 0 else nc.scalar
                dma_eng.dma_start(
                    out=Rt,
                    in_=rf[:, h2 * YB * GW : (h2 + 1) * YB * GW])
                R[h2] = Rt.rearrange("p (g w) -> p g w", w=GW)

            # ---- constants on the SP queue (after features)
            sel = consts.tile([128, 32], FP16)
            nc.sync.dma_start(out=sel, in_=sel_d)
            sid = consts.tile([XB, XB], FP16)
            nc.sync.dma_start(out=sid, in_=sid_d)
            tid = consts.tile([XB, D * XB], FP16)
            nc.sync.dma_start(out=tid, in_=tid_d)
            wxT = [consts.tile([XB, WI], FP16, name=f"wxT{i}", tag=f"wxT{i}")
                   for i in range(2)]
            nc.sync.dma_start(out=wxT[0], in_=wxa_d)
            nc.sync.dma_start(out=wxT[1], in_=wxb_d)
            wyT = consts.tile([128, HI], FP16)
            nc.sync.dma_start(out=wyT, in_=wyT_d)

            st = [stps.tile([XB, 512], F32, name=f"st{h}", tag=f"st{h}")
                  for h in range(SPC)]

            # ---------- copy lanes
            mid_tick = [0]
            tail_tick = [0]

            def _copy_on(eng, dst, src):
                # PSUM->SBUF moves: ACT or DVE only (GPSIMD cannot touch
                # PSUM -- the BIR verifier rejects it)
                if eng == "A":
                    nc.scalar.copy(out=dst, in_=src)
                else:
                    nc.vector.tensor_copy(out=dst, in_=src)

            MID_PAT = list(cfg["mid_pat"])
            TAIL_PAT = list(cfg["tail_pat"])
            DMA_PAT = list(cfg["dma_pat"])
            ENG = {"S": nc.sync, "A": nc.scalar, "P": nc.gpsimd,
                   "V": nc.vector}

            def copy_mid(dst, src):
                _copy_on(MID_PAT[mid_tick[0] % len(MID_PAT)], dst, src)
                mid_tick[0] += 1

            def copy_tail(dst, src):
                _copy_on(TAIL_PAT[tail_tick[0] % len(TAIL_PAT)], dst, src)
                tail_tick[0] += 1

            # ============ software pipeline over the two samples =========
            pred = [None, None]
            # diff tile sections hold disparities hi-first: [d+3,d+2,d+1,d]
            st_open = [False, False]

            def emit_ph1_group(h, d0, nsec, eng=None, split_exp=False,
                               last_g=False):
                eng = eng or nc.vector
                Dt = diffp.tile([128, 4 * YB * W0], FP16, name="diff",
                                tag="diff")
                D4 = Dt.rearrange("p (s g w) -> p s g w", s=4, w=W0)[:, 0:nsec]
                Lk = Lh[h].unsqueeze(1).broadcast_to([128, nsec, YB, W0])
                # one subtract covers disparities d0+nsec-1..d0 via a k-dim
                # stepping the feat_r window right by 1
                off_hi = PAD - (d0 + nsec - 1)
                Rbase = R[h][:, :, off_hi : off_hi + W0]
                Rk = bass.AP(
                    Rbase.tensor, Rbase.offset,
                    [list(Rbase.ap[0]), [1, nsec],
                     list(Rbase.ap[1]), list(Rbase.ap[2])])
                eng.tensor_tensor(out=D4, in0=Lk, in1=Rk, op=OP.subtract)
                Du = Dt.bitcast(U16)[:, 0 : nsec * YB * W0]
                nc.vector.tensor_scalar(
                    out=Du, in0=Du, scalar1=0x7FFF, scalar2=None,
                    op0=OP.bitwise_and,
                )
                D3 = Dt.rearrange("p (s f) -> p s f", s=4)
                # flipped channel sum: cost[x, (sec, yb, xb, y32)]
                cost = costp.tile([XB, 1024], F32, name="cost", tag="cost")
                e = ep.tile([XB, 1024], FP16, name="e", tag="e")
                for sec in range(nsec):
                    for yb in range(YB):
                        for xb in range(2):
                            nc.tensor.matmul(
                                cost[0:XB,
                                     sec * 256 + yb * 64 + xb * 32 :
                                     sec * 256 + yb * 64 + xb * 32 + 32],
                                lhsT=D3[:, sec,
                                        yb * W0 + xb * XB :
                                        yb * W0 + xb * XB + XB],
                                rhs=sel,
                                start=(sec in (0, 2) and yb == 0 and xb == 0),
                                stop=(sec in (1, 3, nsec - 1)
                                      and yb == YB - 1 and xb == 1),
                                skip_group_check=True,
                            )
                ncols = nsec * 256
                if split_exp and nsec > 2:
                    # half-exps emitted back-to-back so the tail chain
                    # [cs -> exp -> st -> pred] is as short as possible
                    for hf in range(2):
                        nc.scalar.activation(
                            out=e[:, hf * ncols // 2 : ncols // 2 * (hf + 1)],
                            in_=cost[:, hf * ncols // 2 :
                                     ncols // 2 * (hf + 1)],
                            func=AF.Exp, bias=bias8, scale=-1.0)
                else:
                    nc.scalar.activation(out=e[:, 0:ncols],
                                         in_=cost[:, 0:ncols],
                                         func=AF.Exp, bias=bias8, scale=-1.0)
                for sec in range(nsec):
                    _emit_st_sec(h, d0 + (nsec - 1 - sec), sec, e, last_g
                                 and sec == nsec - 1)

            def _emit_st_sec(h, d, sec, e, last_sec):
                for yb in range(YB):
                    for xb in range(2):
                        ecol = sec * 256 + yb * 64 + xb * 32
                        scol = xb * 256 + yb * 64
                        first = not st_open[h]
                        st_open[h] = True
                        last = last_sec and yb == YB - 1 and xb == 1
                        rhs = e[:, ecol : ecol + 32]
                        nc.tensor.matmul(
                            st[h][0:XB, scol : scol + 32],
                            lhsT=sid, rhs=rhs,
                            start=first, stop=False,
                            skip_group_check=True,
                        )
                        nc.tensor.matmul(
                            st[h][0:XB, scol + 32 : scol + 64],
                            lhsT=tid[:, d * XB : d * XB + XB], rhs=rhs,
                            start=False, stop=last,
                            skip_group_check=True,
                        )

            def _strips(ap2d, off):
                # [120, 512] -> [120, 8, 32] strips at (yb, xb) stride 64
                return bass.AP(ap2d.tensor, ap2d.offset + off,
                               [list(ap2d.ap[0]), [64, 8], [1, 32]])

            def emit_pred(h):
                pr = predp.tile([XB, 256], FP16, name=f"pred{h}",
                                tag=f"pred{h}")
                rs = predp.tile([XB, 256], F32, name=f"rs{h}", tag=f"rs{h}")
                nc.vector.reciprocal(out=rs, in_=_strips(st[h], 0))
                nc.vector.tensor_tensor(out=pr, in0=_strips(st[h], 32),
                                        in1=rs, op=OP.mult)
                pred[h] = pr

            def emit_ph2_head(h, copy_fn, pool_fns=None):
                """M1 -> (tmpA, tmpB); fat tiles when pool_fns given."""
                pr = pred[h]
                lhs = [pr[:, xb * 128 : xb * 128 + 128]
                       for xb in range(2)]
                tmpA = upsb.tile([128, XSPLIT], FP16, tag=f"tmpA{h}",
                                 name=f"tmpA{h}")
                tmpB = upsb.tile([128, WP - XSPLIT], FP16, tag=f"tmpB{h}",
                                 name=f"tmpB{h}")

                def tmp_slice(base, wtot):
                    if base < XSPLIT:
                        return tmpA[:, base : base + wtot]
                    return tmpB[:, base - XSPLIT : base - XSPLIT + wtot]

                if pool_fns is None:
                    for c0, nw, halves in XCH_M1:
                        t_ps = outps.tile([128, 512], F32, name="o_ps",
                                          tag="o_ps")
                        for i, xb in enumerate(halves):
                            nc.tensor.matmul(
                                t_ps[:, 0:nw], lhsT=lhs[xb],
                                rhs=wxT[xb][:, c0 : c0 + nw],
                                start=(i == 0), stop=(i == len(halves) - 1),
                            )
                        copy_fn(tmp_slice(c0, nw), t_ps[:, 0:nw])
                    return tmpA, tmpB
                for half, (lo, hi, base) in enumerate(
                        ((0, 2, 0), (2, 5, XSPLIT))):
                    t_ps = pool_fns[half]()
                    for c0, nw, halves in XCH_M1[lo:hi]:
                        for i, xb in enumerate(halves):
                            nc.tensor.matmul(
                                t_ps[:, c0 - base : c0 - base + nw],
                                lhsT=lhs[xb],
                                rhs=wxT[xb][:, c0 : c0 + nw],
                                start=(i == 0),
                                stop=(i == len(halves) - 1),
                            )
                    wtot = (XCH_M1[hi - 1][0] + XCH_M1[hi - 1][1]) - base
                    copy_fn(tmp_slice(base, wtot), t_ps[:, 0:wtot])
                return tmpA, tmpB

            def _tmp_rhs(tmp2, c0, nw):
                tmpA, tmpB = tmp2
                if c0 < XSPLIT:
                    return tmpA[:, c0 : c0 + nw]
                return tmpB[:, c0 - XSPLIT : c0 - XSPLIT + nw]

            dma_tick = [0]

            def emit_ph2_yc(h, tmp2, yc, copy_fn, pool_fns=None,
                            dma_engs=None):
                """pool_fns=None: four 1-bank chunks (mid rows, low PSUM
                pressure).  pool_fns: two 2-bank tiles + fat copies +
                half-row DMAs (tail)."""
                ob = outsb.tile([128, WP], FP16, name="ob", tag="ob")
                if pool_fns is None:
                    for c0, nw in XCH:
                        o_ps = outps.tile([128, 512], F32, name="o_ps",
                                          tag="o_ps")
                        nc.tensor.matmul(
                            o_ps[:, 0:nw],
                            lhsT=wyT[:, yc * 128 : yc * 128 + 128],
                            rhs=_tmp_rhs(tmp2, c0, nw),
                            start=True, stop=True,
                        )
                        copy_fn(ob[:, c0 : c0 + nw], o_ps[:, 0:nw])
                    nc.sync.dma_start(
                        out=out[h, yc * 128 : yc * 128 + 128, :], in_=ob)
                    return
                for half, (cs_, cw) in enumerate(
                        ((0, XSPLIT), (XSPLIT, WP - XSPLIT))):
                    o_ps = pool_fns[half]()
                    for ci in range(2):
                        c0, nw = XCH[half * 2 + ci]
                        nc.tensor.matmul(
                            o_ps[:, c0 - cs_ : c0 - cs_ + nw],
                            lhsT=wyT[:, yc * 128 : yc * 128 + 128],
                            rhs=_tmp_rhs(tmp2, c0, nw),
                            start=True, stop=True,
                        )
                    copy_fn(ob[:, cs_ : cs_ + cw], o_ps[:, 0:cw])
                    if dma_engs is not None:
                        eng = dma_engs[half]
                    else:
                        eng = ENG[DMA_PAT[dma_tick[0] % len(DMA_PAT)]]
                        dma_tick[0] += 1
                    eng.dma_start(
                        out=out[h, yc * 128 : yc * 128 + 128,
                                cs_ : cs_ + cw],
                        in_=ob[:, cs_ : cs_ + cw])

            # sample 0 cost volume + regression; Pool takes POOL_G groups.
            # the last two groups are 2 disparities each so the final
            # [abs -> cs -> exp -> st -> pred] chain is short
            GRP = [(0, 4), (4, 4), (8, 4), (12, 4), (16, 4), (20, 2),
                   (22, 2)]
            NG = len(GRP)
            POOL_G0 = set(cfg["pool_g0"])
            POOL_G1 = set(cfg["pool_g1"])
            for g in range(NG):
                emit_ph1_group(0, *GRP[g],
                               eng=nc.gpsimd if g in POOL_G0 else None,
                               split_exp=(g == NG - 1), last_g=(g == NG - 1))
            emit_ph1_group(1, *GRP[0])
            emit_pred(0)
            tmp0 = emit_ph2_head(0, copy_mid)
            for g in range(1, NG):
                emit_ph1_group(1, *GRP[g],
                               eng=nc.gpsimd if g in POOL_G1 else None,
                               split_exp=(g == NG - 1), last_g=(g == NG - 1))
                emit_ph2_yc(0, tmp0, g - 1, copy_mid)   # rows 0..5
            emit_pred(1)
            ph1_stack.close()  # free cost (4) + s/t (2) banks for the tail
            with tc.tile_pool(name="pstail", bufs=3, space="PSUM") as pstail:
                def tail_tile():
                    return pstail.tile([128, 1024], F32, name="tl",
                                       tag="tl")

                tail_pools = (tail_tile, tail_tile)
                emit_ph2_yc(0, tmp0, 6, copy_tail, tail_pools)
                emit_ph2_yc(0, tmp0, 7, copy_tail, tail_pools)
                tmp1 = emit_ph2_head(1, copy_tail, tail_pools)
                for yc in range(8):
                    emit_ph2_yc(1, tmp1, yc, copy_tail, tail_pools)
    nc.compile()
    return nc


_NC_CACHE = [None]


def kernel(feat_l, feat_r, img_h, img_w):
    feat_l = np.asarray(feat_l, dtype=np.float32)
    feat_r = np.asarray(feat_r, dtype=np.float32)
    assert int(img_h) == HI and int(img_w) == WI
    assert feat_l.shape == (B, C, H0, W0)

    if _NC_CACHE[0] is None:
        _NC_CACHE[0] = _build()
    nc = _NC_CACHE[0]

    consts = _host_consts()
    in_maps = []
    for c in range(NCORES):
        fl = _pack_feat(feat_l[SPC * c : SPC * c + SPC].astype(np.float16))
        fr = _pack_feat_padded(
            feat_r[SPC * c : SPC * c + SPC].astype(np.float16))
        in_maps.append({"lf": fl, "rf": fr, **consts})

    res = run_bass_kernel_spmd(nc, in_maps, core_ids=list(range(NCORES)),
                               trace=_TRACE[0])
    outs = [res.results[i]["out"].astype(np.float32) for i in range(NCORES)]
    full = np.concatenate(outs, axis=0).reshape(B, 1, HI, WI)
    kernel._last_exec_ns = res.exec_time_ns
    return full


# revision 66
# speedup vs baseline: 1.2807x; 1.0348x over previous
"""Stereo cost-volume + softmax disparity regression + bilinear upsample.

Full inputs:  feat_l, feat_r [16, 4, 128, 240] f32, img_h=1024, img_w=1920.
Full output:  [16, 1, 1024, 1920] f32.

Sharding: pure data parallel, 2 samples per core across 8 cores; the two
samples run as a software pipeline (sample 1's cost volume overlaps
sample 0's upsample).

Phase 1 (7 disparity groups per sample: 5x4 + 2x2, the small ones last so
the final [abs -> cs -> exp -> st -> pred] chain is short):
  - DVE computes |L - R(x-d)| for a whole group in ONE subtract (custom
    4D access pattern walking the host-pre-padded feat_r window at
    stride 1) + ONE 4x-mode u16 bitwise abs.  Pool (GPSIMD) runs the
    subtract for a few groups in parallel (it cannot run the bitwise
    abs or touch PSUM -- BIR verifier rules -- so DVE abs's those too).
  - Channel sum runs "flipped" on the PE: the diff chunk [128, 120] is
    the stationary lhsT and the [128, 32] selector streams, producing
    cost chunks [120(x), 32(y32)] -- 4x fewer streamed columns than
    streaming the diff.  Layout: cost[x-block, (sec, yb, xb, y32)].
  - ACT exponentiates a whole group tile [120, <=1024] at once.
  - s/t accumulate in PSUM via scaled-identity lhsT matmuls (s += e,
    t += 8d*e) into [120, 32] regions keyed (xb, yb); PSUM lazy-zero
    semantics allow interleaved region accumulation with start exactly
    once per bank (skip_group_check).
Phase 2: pred = t * (1/s) comes out ALREADY x-transposed [120(x), (xb,
  yb, y32)], so M1 (x-interp) consumes per-xb contiguous [120, 128]
  slices as lhsT -- no PE transposes.  M1/M2/output all split X at 956,
  the exact pure-A/pure-B wxT boundary (only an 8-column sliver needs
  both x-halves), so each M2 row-half depends on one M1 copy only.
  PSUM->SBUF copies go to ACT (and DVE in the tail; GPSIMD cannot read
  PSUM); the tail uses 2-bank PSUM tiles with one fat copy per row-half.
  Output rows DMA on the idle SP queue, plus ACT/Pool queues in the
  tail, split at the 956 boundary to overlap drain with the last rows.

All engine assignments (which groups Pool subtracts, copy-lane patterns,
DMA queue pattern) live in CFG, tuned by sweeping CoreSim.
"""
import sys

sys.path.insert(0, "/opt/trn_rl_repo")

import numpy as np

import concourse.bass as bass
import concourse.bacc as bacc
import concourse.tile as tile
import concourse.mybir as mybir
from concourse.bass_utils import run_bass_kernel_spmd

# ---------------------------------------------------------------- constants
B, C, H0, W0 = 16, 4, 128, 240
D = 24             # disparities
NCORES = 8
SPC = B // NCORES  # samples per core = 2
HI, WI = 1024, 1920
WP = WI
XB = 120           # x-block width (two blocks per row)
XSPLIT = 956       # X column where the x-interp flips from wxT half A to B
# M2 / output X chunks (PSUM <= 512 cols each, split at XSPLIT)
XCH = [(0, 512), (512, 444), (956, 512), (1468, 452)]
# M1 X chunks: (start, width, x-halves needed); grouped so chunks 0-1 and
# 2-4 each pack into one 2-bank PSUM tile without bank-straddling writes
XCH_M1 = [(0, 512, (0,)), (512, 444, (0,)), (956, 8, (0, 1)),
          (964, 504, (1,)), (1468, 452, (1,))]
YB = H0 // 32      # 4 y-blocks
G = SPC * YB       # 8 feat groups (sample-major)
FREE = G * W0      # 1920
PAD = 28           # left-pad columns in padded feat_r groups
GW = W0 + 2 * PAD  # padded group width (even)
EXP_BIAS = 8.0

FP16 = mybir.dt.float16
F32 = mybir.dt.float32
U16 = mybir.dt.uint16

_TRACE = [False]


# ------------------------------------------------------------- host weights
def _host_consts():
    # selector for the flipped channel sum: sel[ch*32+y32, y'] = (y32 == y')
    sel = np.zeros((128, 32), np.float16)
    for ch in range(C):
        sel[ch * 32 : (ch + 1) * 32, :] = np.eye(32, dtype=np.float16)

    # s identity and per-disparity t identities (8*d scaling)
    sid = np.eye(XB, dtype=np.float16)
    tid = np.zeros((XB, D * XB), np.float16)
    for d in range(D):
        tid[:, d * XB : (d + 1) * XB] = np.eye(XB, dtype=np.float16) * \
            np.float16(8.0 * d)

    # x-interp weights wxT[x, X], f32 linspace to match jnp rounding
    xs = np.linspace(0.0, W0 - 1.0, WI, dtype=np.float32)
    x0 = np.floor(xs).astype(np.int64)
    x1 = np.minimum(x0 + 1, W0 - 1)
    wx = (xs - x0).astype(np.float32)
    wxT_full = np.zeros((W0, WI), np.float32)
    wxT_full[x0, np.arange(WI)] += 1.0 - wx
    wxT_full[x1, np.arange(WI)] += wx
    # chunk validity: columns left of 956 only use x<120; right of 964 only
    # x>=120; the 8-col sliver uses both
    assert x1[:956].max() <= XB - 1
    assert x0[964:].min() >= XB
    wxTa = wxT_full[0:XB]
    wxTb = wxT_full[XB : 2 * XB]

    # y-interp weights wyT[y, Y]
    ys = np.linspace(0.0, H0 - 1.0, HI, dtype=np.float32)
    y0 = np.floor(ys).astype(np.int64)
    y1 = np.minimum(y0 + 1, H0 - 1)
    wy = (ys - y0).astype(np.float32)
    wyT = np.zeros((H0, HI), np.float32)
    wyT[y0, np.arange(HI)] += 1.0 - wy
    wyT[y1, np.arange(HI)] += wy

    return {
        "sel": sel,
        "sid": sid,
        "tid": tid,
        "wxTa": wxTa.astype(np.float16),
        "wxTb": wxTb.astype(np.float16),
        "wyT": wyT.astype(np.float16),
    }


def _pack_feat(f):
    """[SPC, C, H0, W0] -> [128, FREE] with p=(ch,y32), free=(s,yb,x)."""
    a = f.reshape(SPC, C, YB, 32, W0)
    a = np.ascontiguousarray(a.transpose(1, 3, 0, 2, 4))  # ch,y32,s,yb,x
    return a.reshape(128, FREE)


def _pack_feat_padded(f):
    """[SPC, C, H0, W0] -> [128, SPC*YB*GW], PAD zero cols around each row."""
    a = f.reshape(SPC, C, YB, 32, W0).transpose(1, 3, 0, 2, 4)
    p = np.zeros((C, 32, SPC, YB, GW), f.dtype)
    p[:, :, :, :, PAD : PAD + W0] = a
    return p.reshape(128, SPC * YB * GW)


# scheduling configuration (engine assignment knobs, tuned via sweep)
CFG = {
    "pool_g0": (1, 2, 3),
    "pool_g1": (1, 2, 3),
    "mid_pat": "AAV",
    "tail_pat": "AV",
    "dma_pat": "SPSA",    # tail out-DMA queues: S=SP, A=ACT, P=Pool
    "mult1": "V",         # engine for sample-1 pred multiply
}


# ------------------------------------------------------------- build kernel
def _build(cfg=None):
    cfg = {**CFG, **(cfg or {})}
    nc = bacc.Bacc("TRN2", target_bir_lowering=False, debug=False,
                   num_devices=NCORES)
    lf = nc.dram_tensor("lf", [128, FREE], FP16, kind="ExternalInput").ap()
    rf = nc.dram_tensor("rf", [128, SPC * YB * GW], FP16,
                        kind="ExternalInput").ap()
    sel_d = nc.dram_tensor("sel", [128, 32], FP16, kind="ExternalInput").ap()
    sid_d = nc.dram_tensor("sid", [XB, XB], FP16, kind="ExternalInput").ap()
    tid_d = nc.dram_tensor("tid", [XB, D * XB], FP16,
                           kind="ExternalInput").ap()
    wxa_d = nc.dram_tensor("wxTa", [XB, WI], FP16, kind="ExternalInput").ap()
    wxb_d = nc.dram_tensor("wxTb", [XB, WI], FP16, kind="ExternalInput").ap()
    wyT_d = nc.dram_tensor("wyT", [H0, HI], FP16, kind="ExternalInput").ap()
    out = nc.dram_tensor("out", [SPC, HI, WI], FP16,
                         kind="ExternalOutput").ap()

    AF = mybir.ActivationFunctionType
    OP = mybir.AluOpType

    with tile.TileContext(nc) as tc:
        with (
            tc.tile_pool(name="consts", bufs=1) as consts,
            tc.tile_pool(name="feat", bufs=1) as feat,
            tc.tile_pool(name="diff", bufs=4) as diffp,
            tc.tile_pool(name="ep", bufs=4) as ep,
            tc.tile_pool(name="predp", bufs=1) as predp,
            tc.tile_pool(name="upsb", bufs=1) as upsb,
            tc.tile_pool(name="outsb", bufs=6) as outsb,
            tc.tile_pool(name="outps", bufs=2, space="PSUM") as outps,
        ):
            from contextlib import ExitStack
            ph1_stack = ExitStack()
            costp = ph1_stack.enter_context(
                tc.tile_pool(name="costp", bufs=2, space="PSUM"))
            stps = ph1_stack.enter_context(
                tc.tile_pool(name="stps", bufs=1, space="PSUM"))

            bias8 = consts.tile([XB, 1], F32)
            nc.vector.memset(bias8, EXP_BIAS)

            # ---- features: left halves on SP; padded right h0 on Pool
            # (ahead of ACT's table load), h1 on ACT.
            # per-sample L tiles so sample-0's subs don't wait on h1's DMA
            Lh = []
            for h2 in range(SPC):
                Lt = feat.tile([128, FREE // 2], FP16, tag=f"L{h2}",
                               name=f"L{h2}")
                nc.sync.dma_start(
                    out=Lt,
                    in_=lf[:, h2 * (FREE // 2) : (h2 + 1) * (FREE // 2)])
                Lh.append(Lt.rearrange("p (g w) -> p g w", w=W0))
            R = [None, None]  # R[h] -> [128, YB, GW]
            for h2 in range(SPC):
                Rt = feat.tile([128, YB * GW], FP16,
                               tag=f"rpad{h2}", name=f"rpad{h2}")
                dma_eng = nc.gpsimd if h2 == 0 else nc.scalar
                dma_eng.dma_start(
                    out=Rt,
                    in_=rf[:, h2 * YB * GW : (h2 + 1) * YB * GW])
                R[h2] = Rt.rearrange("p (g w) -> p g w", w=GW)

            # ---- constants on the SP queue (after features)
            sel = consts.tile([128, 32], FP16)
            nc.sync.dma_start(out=sel, in_=sel_d)
            sid = consts.tile([XB, XB], FP16)
            nc.sync.dma_start(out=sid, in_=sid_d)
            tid = consts.tile([XB, D * XB], FP16)
            nc.sync.dma_start(out=tid, in_=tid_d)
            wxT = [consts.tile([XB, WI], FP16, name=f"wxT{i}", tag=f"wxT{i}")
                   for i in range(2)]
            nc.sync.dma_start(out=wxT[0], in_=wxa_d)
            nc.sync.dma_start(out=wxT[1], in_=wxb_d)
            wyT = consts.tile([128, HI], FP16)
            nc.sync.dma_start(out=wyT, in_=wyT_d)

            st = [stps.tile([XB, 512], F32, name=f"st{h}", tag=f"st{h}")
                  for h in range(SPC)]

            # ---------- copy lanes
            mid_tick = [0]
            tail_tick = [0]

            def _copy_on(eng, dst, src):
                # PSUM->SBUF moves: ACT or DVE only (GPSIMD cannot touch
                # PSUM -- the BIR verifier rejects it)
                if eng == "A":
                    nc.scalar.copy(out=dst, in_=src)
                else:
                    nc.vector.tensor_copy(out=dst, in_=src)

            MID_PAT = list(cfg["mid_pat"])
            TAIL_PAT = list(cfg["tail_pat"])
            DMA_PAT = list(cfg["dma_pat"])
            ENG = {"S": nc.sync, "A": nc.scalar, "P": nc.gpsimd,
                   "V": nc.vector}

            def copy_mid(dst, src):
                _copy_on(MID_PAT[mid_tick[0] % len(MID_PAT)], dst, src)
                mid_tick[0] += 1

            def copy_tail(dst, src):
                _copy_on(TAIL_PAT[tail_tick[0] % len(TAIL_PAT)], dst, src)
                tail_tick[0] += 1

            # ============ software pipeline over the two samples =========
            pred = [None, None]
            # diff tile sections hold disparities hi-first: [d+3,d+2,d+1,d]
            st_open = [False, False]

            def emit_ph1_group(h, d0, nsec, eng=None, split_exp=False,
                               last_g=False):
                eng = eng or nc.vector
                Dt = diffp.tile([128, 4 * YB * W0], FP16, name="diff",
                                tag="diff")
                D4 = Dt.rearrange("p (s g w) -> p s g w", s=4, w=W0)[:, 0:nsec]
                Lk = Lh[h].unsqueeze(1).broadcast_to([128, nsec, YB, W0])
                # one subtract covers disparities d0+nsec-1..d0 via a k-dim
                # stepping the feat_r window right by 1
                off_hi = PAD - (d0 + nsec - 1)
                Rbase = R[h][:, :, off_hi : off_hi + W0]
                Rk = bass.AP(
                    Rbase.tensor, Rbase.offset,
                    [list(Rbase.ap[0]), [1, nsec],
                     list(Rbase.ap[1]), list(Rbase.ap[2])])
                eng.tensor_tensor(out=D4, in0=Lk, in1=Rk, op=OP.subtract)
                Du = Dt.bitcast(U16)[:, 0 : nsec * YB * W0]
                nc.vector.tensor_scalar(
                    out=Du, in0=Du, scalar1=0x7FFF, scalar2=None,
                    op0=OP.bitwise_and,
                )
                D3 = Dt.rearrange("p (s f) -> p s f", s=4)
                # flipped channel sum: cost[x, (sec, yb, xb, y32)]
                cost = costp.tile([XB, 1024], F32, name="cost", tag="cost")
                e = ep.tile([XB, 1024], FP16, name="e", tag="e")
                for sec in range(nsec):
                    for yb in range(YB):
                        for xb in range(2):
                            nc.tensor.matmul(
                                cost[0:XB,
                                     sec * 256 + yb * 64 + xb * 32 :
                                     sec * 256 + yb * 64 + xb * 32 + 32],
                                lhsT=D3[:, sec,
                                        yb * W0 + xb * XB :
                                        yb * W0 + xb * XB + XB],
                                rhs=sel,
                                start=(sec in (0, 2) and yb == 0 and xb == 0),
                                stop=(sec in (1, 3, nsec - 1)
                                      and yb == YB - 1 and xb == 1),
                                skip_group_check=True,
                            )
                ncols = nsec * 256
                if split_exp and nsec > 2:
                    # half-exps emitted back-to-back so the tail chain
                    # [cs -> exp -> st -> pred] is as short as possible
                    for hf in range(2):
                        nc.scalar.activation(
                            out=e[:, hf * ncols // 2 : ncols // 2 * (hf + 1)],
                            in_=cost[:, hf * ncols // 2 :
                                     ncols // 2 * (hf + 1)],
                            func=AF.Exp, bias=bias8, scale=-1.0)
                else:
                    nc.scalar.activation(out=e[:, 0:ncols],
                                         in_=cost[:, 0:ncols],
                                         func=AF.Exp, bias=bias8, scale=-1.0)
                for sec in range(nsec):
                    _emit_st_sec(h, d0 + (nsec - 1 - sec), sec, e, last_g
                                 and sec == nsec - 1)

            def _emit_st_sec(h, d, sec, e, last_sec):
                for yb in range(YB):
                    for xb in range(2):
                        ecol = sec * 256 + yb * 64 + xb * 32
                        scol = xb * 256 + yb * 64
                        first = not st_open[h]
                        st_open[h] = True
                        last = last_sec and yb == YB - 1 and xb == 1
                        rhs = e[:, ecol : ecol + 32]
                        nc.tensor.matmul(
                            st[h][0:XB, scol : scol + 32],
                            lhsT=sid, rhs=rhs,
                            start=first, stop=False,
                            skip_group_check=True,
                        )
                        nc.tensor.matmul(
                            st[h][0:XB, scol + 32 : scol + 64],
                            lhsT=tid[:, d * XB : d * XB + XB], rhs=rhs,
                            start=False, stop=last,
                            skip_group_check=True,
                        )

            def _strips(ap2d, off):
                # [120, 512] -> [120, 8, 32] strips at (yb, xb) stride 64
                return bass.AP(ap2d.tensor, ap2d.offset + off,
                               [list(ap2d.ap[0]), [64, 8], [1, 32]])

            def emit_pred(h):
                pr = predp.tile([XB, 256], FP16, name=f"pred{h}",
                                tag=f"pred{h}")
                rs = predp.tile([XB, 256], F32, name=f"rs{h}", tag=f"rs{h}")
                nc.vector.reciprocal(out=rs, in_=_strips(st[h], 0))
                nc.vector.tensor_tensor(out=pr, in0=_strips(st[h], 32),
                                        in1=rs, op=OP.mult)
                pred[h] = pr

            def emit_ph2_head(h, copy_fn, pool_fns=None):
                """M1 -> (tmpA, tmpB); fat tiles when pool_fns given."""
                pr = pred[h]
                lhs = [pr[:, xb * 128 : xb * 128 + 128]
                       for xb in range(2)]
                tmpA = upsb.tile([128, XSPLIT], FP16, tag=f"tmpA{h}",
                                 name=f"tmpA{h}")
                tmpB = upsb.tile([128, WP - XSPLIT], FP16, tag=f"tmpB{h}",
                                 name=f"tmpB{h}")

                def tmp_slice(base, wtot):
                    if base < XSPLIT:
                        return tmpA[:, base : base + wtot]
                    return tmpB[:, base - XSPLIT : base - XSPLIT + wtot]

                if pool_fns is None:
                    for c0, nw, halves in XCH_M1:
                        t_ps = outps.tile([128, 512], F32, name="o_ps",
                                          tag="o_ps")
                        for i, xb in enumerate(halves):
                            nc.tensor.matmul(
                                t_ps[:, 0:nw], lhsT=lhs[xb],
                                rhs=wxT[xb][:, c0 : c0 + nw],
                                start=(i == 0), stop=(i == len(halves) - 1),
                            )
                        copy_fn(tmp_slice(c0, nw), t_ps[:, 0:nw])
                    return tmpA, tmpB
                for half, (lo, hi, base) in enumerate(
                        ((0, 2, 0), (2, 5, XSPLIT))):
                    t_ps = pool_fns[half]()
                    for c0, nw, halves in XCH_M1[lo:hi]:
                        for i, xb in enumerate(halves):
                            nc.tensor.matmul(
                                t_ps[:, c0 - base : c0 - base + nw],
                                lhsT=lhs[xb],
                                rhs=wxT[xb][:, c0 : c0 + nw],
                                start=(i == 0),
                                stop=(i == len(halves) - 1),
                            )
                    wtot = (XCH_M1[hi - 1][0] + XCH_M1[hi - 1][1]) - base
                    copy_fn(tmp_slice(base, wtot), t_ps[:, 0:wtot])
                return tmpA, tmpB

            def _tmp_rhs(tmp2, c0, nw):
                tmpA, tmpB = tmp2
                if c0 < XSPLIT:
                    return tmpA[:, c0 : c0 + nw]
                return tmpB[:, c0 - XSPLIT : c0 - XSPLIT + nw]

            dma_tick = [0]

            def emit_ph2_yc(h, tmp2, yc, copy_fn, pool_fns=None,
                            dma_engs=None):
                """pool_fns=None: four 1-bank chunks (mid rows, low PSUM
                pressure).  pool_fns: two 2-bank tiles + fat copies +
                half-row DMAs (tail)."""
                ob = outsb.tile([128, WP], FP16, name="ob", tag="ob")
                if pool_fns is None:
                    for c0, nw in XCH:
                        o_ps = outps.tile([128, 512], F32, name="o_ps",
                                          tag="o_ps")
                        nc.tensor.matmul(
                            o_ps[:, 0:nw],
                            lhsT=wyT[:, yc * 128 : yc * 128 + 128],
                            rhs=_tmp_rhs(tmp2, c0, nw),
                            start=True, stop=True,
                        )
                        copy_fn(ob[:, c0 : c0 + nw], o_ps[:, 0:nw])
                    nc.sync.dma_start(
                        out=out[h, yc * 128 : yc * 128 + 128, :], in_=ob)
                    return
                for half, (cs_, cw) in enumerate(
                        ((0, XSPLIT), (XSPLIT, WP - XSPLIT))):
                    o_ps = pool_fns[half]()
                    for ci in range(2):
                        c0, nw = XCH[half * 2 + ci]
                        nc.tensor.matmul(
                            o_ps[:, c0 - cs_ : c0 - cs_ + nw],
                            lhsT=wyT[:, yc * 128 : yc * 128 + 128],
                            rhs=_tmp_rhs(tmp2, c0, nw),
                            start=True, stop=True,
                        )
                    copy_fn(ob[:, cs_ : cs_ + cw], o_ps[:, 0:cw])
                    if dma_engs is not None:
                        eng = dma_engs[half]
                    else:
                        eng = ENG[DMA_PAT[dma_tick[0] % len(DMA_PAT)]]
                        dma_tick[0] += 1
                    eng.dma_start(
                        out=out[h, yc * 128 : yc * 128 + 128,
                                cs_ : cs_ + cw],
                        in_=ob[:, cs_ : cs_ + cw])

            # sample 0 cost volume + regression; Pool takes POOL_G groups.
            # the last two groups are 2 disparities each so the final
            # [abs -> cs -> exp -> st -> pred] chain is short
            GRP = [(0, 4), (4, 4), (8, 4), (12, 4), (16, 4), (20, 2),
                   (22, 2)]
            NG = len(GRP)
            POOL_G0 = set(cfg["pool_g0"])
            POOL_G1 = set(cfg["pool_g1"])
            for g in range(NG):
                emit_ph1_group(0, *GRP[g],
                               eng=nc.gpsimd if g in POOL_G0 else None,
                               split_exp=(g == NG - 1), last_g=(g == NG - 1))
            emit_ph1_group(1, *GRP[0])
            emit_pred(0)
            tmp0 = emit_ph2_head(0, copy_mid)
            for g in range(1, NG):
                emit_ph1_group(1, *GRP[g],
                               eng=nc.gpsimd if g in POOL_G1 else None,
                               split_exp=(g == NG - 1), last_g=(g == NG - 1))
                emit_ph2_yc(0, tmp0, g - 1, copy_mid)   # rows 0..5
            emit_pred(1)
            ph1_stack.close()  # free cost (4) + s/t (2) banks for the tail
            with tc.tile_pool(name="pstail", bufs=3, space="PSUM") as pstail:
                def tail_tile():
                    return pstail.tile([128, 1024], F32, name="tl",
                                       tag="tl")

                tail_pools = (tail_tile, tail_tile)
                emit_ph2_yc(0, tmp0, 6, copy_tail, tail_pools)
                emit_ph2_yc(0, tmp0, 7, copy_tail, tail_pools)
                tmp1 = emit_ph2_head(1, copy_tail, tail_pools)
                for yc in range(8):
                    emit_ph2_yc(1, tmp1, yc, copy_tail, tail_pools)
    nc.compile()
    return nc


_NC_CACHE = [None]


def kernel(feat_l, feat_r, img_h, img_w):
    feat_l = np.asarray(feat_l, dtype=np.float32)
    feat_r = np.asarray(feat_r, dtype=np.float32)
    assert int(img_h) == HI and int(img_w) == WI
    assert feat_l.shape == (B, C, H0, W0)

    if _NC_CACHE[0] is None:
        _NC_CACHE[0] = _build()
    nc = _NC_CACHE[0]

    consts = _host_consts()
    in_maps = []
    for c in range(NCORES):
        fl = _pack_feat(feat_l[SPC * c : SPC * c + SPC].astype(np.float16))
        fr = _pack_feat_padded(
            feat_r[SPC * c : SPC * c + SPC].astype(np.float16))
        in_maps.append({"lf": fl, "rf": fr, **consts})

    res = run_bass_kernel_spmd(nc, in_maps, core_ids=list(range(NCORES)),
                               trace=_TRACE[0])
    outs = [res.results[i]["out"].astype(np.float32) for i in range(NCORES)]
    full = np.concatenate(outs, axis=0).reshape(B, 1, HI, WI)
    kernel._last_exec_ns = res.exec_time_ns
    return full


# revision 67
# speedup vs baseline: 1.3218x; 1.0320x over previous
"""Stereo cost-volume + softmax disparity regression + bilinear upsample.

Full inputs:  feat_l, feat_r [16, 4, 128, 240] f32, img_h=1024, img_w=1920.
Full output:  [16, 1, 1024, 1920] f32.

Sharding: pure data parallel, 2 samples per core across 8 cores; the two
samples run as a software pipeline (sample 1's cost volume overlaps
sample 0's upsample).

Phase 1 (7 disparity groups per sample: 5x4 + 2x2, the small ones last so
the final [abs -> cs -> exp -> st -> pred] chain is short):
  - DVE computes |L - R(x-d)| for a whole group in ONE subtract (custom
    4D access pattern walking the host-pre-padded feat_r window at
    stride 1) + ONE 4x-mode u16 bitwise abs.  Pool (GPSIMD) runs the
    subtract for a few groups in parallel (it cannot run the bitwise
    abs or touch PSUM -- BIR verifier rules -- so DVE abs's those too).
  - Channel sum runs "flipped" on the PE: the diff chunk [128, 120] is
    the stationary lhsT and the [128, 32] selector streams, producing
    cost chunks [120(x), 32(y32)] -- 4x fewer streamed columns than
    streaming the diff.  Layout: cost[x-block, (sec, yb, xb, y32)].
  - ACT exponentiates a whole group tile [120, <=1024] at once.
  - s/t accumulate in PSUM via scaled-identity lhsT matmuls (s += e,
    t += 8d*e) into [120, 32] regions keyed (xb, yb); PSUM lazy-zero
    semantics allow interleaved region accumulation with start exactly
    once per bank (skip_group_check).
Phase 2: pred = t * (1/s) comes out ALREADY x-transposed [120(x), (xb,
  yb, y32)], so M1 (x-interp) consumes per-xb contiguous [120, 128]
  slices as lhsT -- no PE transposes.  M1/M2/output all split X at 956,
  the exact pure-A/pure-B wxT boundary (only an 8-column sliver needs
  both x-halves), so each M2 row-half depends on one M1 copy only.
  PSUM->SBUF copies go to ACT (and DVE in the tail; GPSIMD cannot read
  PSUM); the tail uses 2-bank PSUM tiles with one fat copy per row-half.
  Output rows DMA on the idle SP queue, plus ACT/Pool queues in the
  tail, split at the 956 boundary to overlap drain with the last rows.

All engine assignments (which groups Pool subtracts, copy-lane patterns,
DMA queue pattern) live in CFG, tuned by sweeping CoreSim.
"""
import sys

sys.path.insert(0, "/opt/trn_rl_repo")

import numpy as np

import concourse.bass as bass
import concourse.bacc as bacc
import concourse.tile as tile
import concourse.mybir as mybir
from concourse.bass_utils import run_bass_kernel_spmd

# ---------------------------------------------------------------- constants
B, C, H0, W0 = 16, 4, 128, 240
D = 24             # disparities
NCORES = 8
SPC = B // NCORES  # samples per core = 2
HI, WI = 1024, 1920
WP = WI
XB = 120           # x-block width (two blocks per row)
XSPLIT = 956       # X column where the x-interp flips from wxT half A to B
# M2 / output X chunks (PSUM <= 512 cols each, split at XSPLIT)
XCH = [(0, 512), (512, 444), (956, 512), (1468, 452)]
# M1 X chunks: (start, width, x-halves needed); grouped so chunks 0-1 and
# 2-4 each pack into one 2-bank PSUM tile without bank-straddling writes
XCH_M1 = [(0, 512, (0,)), (512, 444, (0,)), (956, 8, (0, 1)),
          (964, 504, (1,)), (1468, 452, (1,))]
YB = H0 // 32      # 4 y-blocks
G = SPC * YB       # 8 feat groups (sample-major)
FREE = G * W0      # 1920
PAD = 28           # left-pad columns in padded feat_r groups
GW = W0 + 2 * PAD  # padded group width (even)
EXP_BIAS = 8.0

FP16 = mybir.dt.float16
F32 = mybir.dt.float32
U16 = mybir.dt.uint16

_TRACE = [False]


# ------------------------------------------------------------- host weights
def _host_consts():
    # selector for the flipped channel sum: sel[ch*32+y32, y'] = (y32 == y')
    sel = np.zeros((128, 32), np.float16)
    for ch in range(C):
        sel[ch * 32 : (ch + 1) * 32, :] = np.eye(32, dtype=np.float16)

    # s identity and per-disparity t identities (8*d scaling)
    sid = np.eye(XB, dtype=np.float16)
    tid = np.zeros((XB, D * XB), np.float16)
    for d in range(D):
        tid[:, d * XB : (d + 1) * XB] = np.eye(XB, dtype=np.float16) * \
            np.float16(8.0 * d)

    # x-interp weights wxT[x, X], f32 linspace to match jnp rounding
    xs = np.linspace(0.0, W0 - 1.0, WI, dtype=np.float32)
    x0 = np.floor(xs).astype(np.int64)
    x1 = np.minimum(x0 + 1, W0 - 1)
    wx = (xs - x0).astype(np.float32)
    wxT_full = np.zeros((W0, WI), np.float32)
    wxT_full[x0, np.arange(WI)] += 1.0 - wx
    wxT_full[x1, np.arange(WI)] += wx
    # chunk validity: columns left of 956 only use x<120; right of 964 only
    # x>=120; the 8-col sliver uses both
    assert x1[:956].max() <= XB - 1
    assert x0[964:].min() >= XB
    wxTa = wxT_full[0:XB]
    wxTb = wxT_full[XB : 2 * XB]

    # y-interp weights wyT[y, Y]
    ys = np.linspace(0.0, H0 - 1.0, HI, dtype=np.float32)
    y0 = np.floor(ys).astype(np.int64)
    y1 = np.minimum(y0 + 1, H0 - 1)
    wy = (ys - y0).astype(np.float32)
    wyT = np.zeros((H0, HI), np.float32)
    wyT[y0, np.arange(HI)] += 1.0 - wy
    wyT[y1, np.arange(HI)] += wy

    return {
        "sel": sel,
        "sid": sid,
        "tid": tid,
        "wxTa": wxTa.astype(np.float16),
        "wxTb": wxTb.astype(np.float16),
        "wyT": wyT.astype(np.float16),
    }


def _pack_feat(f):
    """[SPC, C, H0, W0] -> [128, FREE] with p=(ch,y32), free=(s,yb,x)."""
    a = f.reshape(SPC, C, YB, 32, W0)
    a = np.ascontiguousarray(a.transpose(1, 3, 0, 2, 4))  # ch,y32,s,yb,x
    return a.reshape(128, FREE)


def _pack_feat_padded(f):
    """[SPC, C, H0, W0] -> [128, SPC*YB*GW], PAD zero cols around each row."""
    a = f.reshape(SPC, C, YB, 32, W0).transpose(1, 3, 0, 2, 4)
    p = np.zeros((C, 32, SPC, YB, GW), f.dtype)
    p[:, :, :, :, PAD : PAD + W0] = a
    return p.reshape(128, SPC * YB * GW)


# scheduling configuration (engine assignment knobs, tuned via sweep)
CFG = {
    "pool_g0": (1, 2, 3),
    "pool_g1": (1, 2, 3),
    "mid_pat": "AAV",
    "tail_pat": "AV",
    "dma_pat": "PS",    # tail out-DMA queues: S=SP, A=ACT, P=Pool
    "mult1": "V",         # engine for sample-1 pred multiply
}


# ------------------------------------------------------------- build kernel
def _build(cfg=None):
    cfg = {**CFG, **(cfg or {})}
    nc = bacc.Bacc("TRN2", target_bir_lowering=False, debug=False,
                   num_devices=NCORES)
    lf = nc.dram_tensor("lf", [128, FREE], FP16, kind="ExternalInput").ap()
    rf = nc.dram_tensor("rf", [128, SPC * YB * GW], FP16,
                        kind="ExternalInput").ap()
    sel_d = nc.dram_tensor("sel", [128, 32], FP16, kind="ExternalInput").ap()
    sid_d = nc.dram_tensor("sid", [XB, XB], FP16, kind="ExternalInput").ap()
    tid_d = nc.dram_tensor("tid", [XB, D * XB], FP16,
                           kind="ExternalInput").ap()
    wxa_d = nc.dram_tensor("wxTa", [XB, WI], FP16, kind="ExternalInput").ap()
    wxb_d = nc.dram_tensor("wxTb", [XB, WI], FP16, kind="ExternalInput").ap()
    wyT_d = nc.dram_tensor("wyT", [H0, HI], FP16, kind="ExternalInput").ap()
    out = nc.dram_tensor("out", [SPC, HI, WI], FP16,
                         kind="ExternalOutput").ap()

    AF = mybir.ActivationFunctionType
    OP = mybir.AluOpType

    with tile.TileContext(nc) as tc:
        with (
            tc.tile_pool(name="consts", bufs=1) as consts,
            tc.tile_pool(name="feat", bufs=1) as feat,
            tc.tile_pool(name="diff", bufs=4) as diffp,
            tc.tile_pool(name="ep", bufs=4) as ep,
            tc.tile_pool(name="predp", bufs=1) as predp,
            tc.tile_pool(name="upsb", bufs=1) as upsb,
            tc.tile_pool(name="outsb", bufs=6) as outsb,
            tc.tile_pool(name="outps", bufs=2, space="PSUM") as outps,
        ):
            from contextlib import ExitStack
            ph1_stack = ExitStack()
            costp = ph1_stack.enter_context(
                tc.tile_pool(name="costp", bufs=2, space="PSUM"))
            stps = ph1_stack.enter_context(
                tc.tile_pool(name="stps", bufs=1, space="PSUM"))

            bias8 = consts.tile([XB, 1], F32)
            nc.vector.memset(bias8, EXP_BIAS)

            # ---- features: left halves on SP; padded right h0 on Pool
            # (ahead of ACT's table load), h1 on ACT.
            # per-sample L tiles so sample-0's subs don't wait on h1's DMA
            Lh = []
            for h2 in range(SPC):
                Lt = feat.tile([128, FREE // 2], FP16, tag=f"L{h2}",
                               name=f"L{h2}")
                nc.sync.dma_start(
                    out=Lt,
                    in_=lf[:, h2 * (FREE // 2) : (h2 + 1) * (FREE // 2)])
                Lh.append(Lt.rearrange("p (g w) -> p g w", w=W0))
            R = [None, None]  # R[h] -> [128, YB, GW]
            for h2 in range(SPC):
                Rt = feat.tile([128, YB * GW], FP16,
                               tag=f"rpad{h2}", name=f"rpad{h2}")
                dma_eng = nc.gpsimd if h2 == 0 else nc.scalar
                dma_eng.dma_start(
                    out=Rt,
                    in_=rf[:, h2 * YB * GW : (h2 + 1) * YB * GW])
                R[h2] = Rt.rearrange("p (g w) -> p g w", w=GW)

            # ---- constants on the SP queue (after features)
            sel = consts.tile([128, 32], FP16)
            nc.sync.dma_start(out=sel, in_=sel_d)
            sid = consts.tile([XB, XB], FP16)
            nc.sync.dma_start(out=sid, in_=sid_d)
            tid = consts.tile([XB, D * XB], FP16)
            nc.sync.dma_start(out=tid, in_=tid_d)
            wxT = [consts.tile([XB, WI], FP16, name=f"wxT{i}", tag=f"wxT{i}")
                   for i in range(2)]
            nc.sync.dma_start(out=wxT[0], in_=wxa_d)
            nc.sync.dma_start(out=wxT[1], in_=wxb_d)
            wyT = consts.tile([128, HI], FP16)
            nc.sync.dma_start(out=wyT, in_=wyT_d)

            st = [stps.tile([XB, 512], F32, name=f"st{h}", tag=f"st{h}")
                  for h in range(SPC)]

            # ---------- copy lanes
            mid_tick = [0]
            tail_tick = [0]

            def _copy_on(eng, dst, src):
                # PSUM->SBUF moves: ACT or DVE only (GPSIMD cannot touch
                # PSUM -- the BIR verifier rejects it)
                if eng == "A":
                    nc.scalar.copy(out=dst, in_=src)
                else:
                    nc.vector.tensor_copy(out=dst, in_=src)

            MID_PAT = list(cfg["mid_pat"])
            TAIL_PAT = list(cfg["tail_pat"])
            DMA_PAT = list(cfg["dma_pat"])
            ENG = {"S": nc.sync, "A": nc.scalar, "P": nc.gpsimd,
                   "V": nc.vector}

            def copy_mid(dst, src):
                _copy_on(MID_PAT[mid_tick[0] % len(MID_PAT)], dst, src)
                mid_tick[0] += 1

            def copy_tail(dst, src):
                _copy_on(TAIL_PAT[tail_tick[0] % len(TAIL_PAT)], dst, src)
                tail_tick[0] += 1

            # ============ software pipeline over the two samples =========
            pred = [None, None]
            # diff tile sections hold disparities hi-first: [d+3,d+2,d+1,d]
            st_open = [False, False]

            def emit_ph1_group(h, d0, nsec, eng=None, split_exp=False,
                               last_g=False):
                eng = eng or nc.vector
                Dt = diffp.tile([128, 4 * YB * W0], FP16, name="diff",
                                tag="diff")
                D4 = Dt.rearrange("p (s g w) -> p s g w", s=4, w=W0)[:, 0:nsec]
                Lk = Lh[h].unsqueeze(1).broadcast_to([128, nsec, YB, W0])
                # one subtract covers disparities d0+nsec-1..d0 via a k-dim
                # stepping the feat_r window right by 1
                off_hi = PAD - (d0 + nsec - 1)
                Rbase = R[h][:, :, off_hi : off_hi + W0]
                Rk = bass.AP(
                    Rbase.tensor, Rbase.offset,
                    [list(Rbase.ap[0]), [1, nsec],
                     list(Rbase.ap[1]), list(Rbase.ap[2])])
                eng.tensor_tensor(out=D4, in0=Lk, in1=Rk, op=OP.subtract)
                Du = Dt.bitcast(U16)[:, 0 : nsec * YB * W0]
                nc.vector.tensor_scalar(
                    out=Du, in0=Du, scalar1=0x7FFF, scalar2=None,
                    op0=OP.bitwise_and,
                )
                D3 = Dt.rearrange("p (s f) -> p s f", s=4)
                # flipped channel sum: cost[x, (sec, yb, xb, y32)]
                cost = costp.tile([XB, 1024], F32, name="cost", tag="cost")
                e = ep.tile([XB, 1024], FP16, name="e", tag="e")
                for sec in range(nsec):
                    for yb in range(YB):
                        for xb in range(2):
                            nc.tensor.matmul(
                                cost[0:XB,
                                     sec * 256 + yb * 64 + xb * 32 :
                                     sec * 256 + yb * 64 + xb * 32 + 32],
                                lhsT=D3[:, sec,
                                        yb * W0 + xb * XB :
                                        yb * W0 + xb * XB + XB],
                                rhs=sel,
                                start=(sec in (0, 2) and yb == 0 and xb == 0),
                                stop=(sec in (1, 3, nsec - 1)
                                      and yb == YB - 1 and xb == 1),
                                skip_group_check=True,
                            )
                ncols = nsec * 256
                if split_exp and nsec > 2:
                    # half-exps emitted back-to-back so the tail chain
                    # [cs -> exp -> st -> pred] is as short as possible
                    for hf in range(2):
                        nc.scalar.activation(
                            out=e[:, hf * ncols // 2 : ncols // 2 * (hf + 1)],
                            in_=cost[:, hf * ncols // 2 :
                                     ncols // 2 * (hf + 1)],
                            func=AF.Exp, bias=bias8, scale=-1.0)
                else:
                    nc.scalar.activation(out=e[:, 0:ncols],
                                         in_=cost[:, 0:ncols],
                                         func=AF.Exp, bias=bias8, scale=-1.0)
                for sec in range(nsec):
                    _emit_st_sec(h, d0 + (nsec - 1 - sec), sec, e, last_g
                                 and sec == nsec - 1)

            def _emit_st_sec(h, d, sec, e, last_sec):
                for yb in range(YB):
                    for xb in range(2):
                        ecol = sec * 256 + yb * 64 + xb * 32
                        scol = xb * 256 + yb * 64
                        first = not st_open[h]
                        st_open[h] = True
                        last = last_sec and yb == YB - 1 and xb == 1
                        rhs = e[:, ecol : ecol + 32]
                        nc.tensor.matmul(
                            st[h][0:XB, scol : scol + 32],
                            lhsT=sid, rhs=rhs,
                            start=first, stop=False,
                            skip_group_check=True,
                        )
                        nc.tensor.matmul(
                            st[h][0:XB, scol + 32 : scol + 64],
                            lhsT=tid[:, d * XB : d * XB + XB], rhs=rhs,
                            start=False, stop=last,
                            skip_group_check=True,
                        )

            def _strips(ap2d, off):
                # [120, 512] -> [120, 8, 32] strips at (yb, xb) stride 64
                return bass.AP(ap2d.tensor, ap2d.offset + off,
                               [list(ap2d.ap[0]), [64, 8], [1, 32]])

            def emit_pred(h):
                pr = predp.tile([XB, 256], FP16, name=f"pred{h}",
                                tag=f"pred{h}")
                rs = predp.tile([XB, 256], F32, name=f"rs{h}", tag=f"rs{h}")
                nc.vector.reciprocal(out=rs, in_=_strips(st[h], 0))
                nc.vector.tensor_tensor(out=pr, in0=_strips(st[h], 32),
                                        in1=rs, op=OP.mult)
                pred[h] = pr

            def emit_ph2_head(h, copy_fn, pool_fns=None):
                """M1 -> (tmpA, tmpB); fat tiles when pool_fns given."""
                pr = pred[h]
                lhs = [pr[:, xb * 128 : xb * 128 + 128]
                       for xb in range(2)]
                tmpA = upsb.tile([128, XSPLIT], FP16, tag=f"tmpA{h}",
                                 name=f"tmpA{h}")
                tmpB = upsb.tile([128, WP - XSPLIT], FP16, tag=f"tmpB{h}",
                                 name=f"tmpB{h}")

                def tmp_slice(base, wtot):
                    if base < XSPLIT:
                        return tmpA[:, base : base + wtot]
                    return tmpB[:, base - XSPLIT : base - XSPLIT + wtot]

                if pool_fns is None:
                    for c0, nw, halves in XCH_M1:
                        t_ps = outps.tile([128, 512], F32, name="o_ps",
                                          tag="o_ps")
                        for i, xb in enumerate(halves):
                            nc.tensor.matmul(
                                t_ps[:, 0:nw], lhsT=lhs[xb],
                                rhs=wxT[xb][:, c0 : c0 + nw],
                                start=(i == 0), stop=(i == len(halves) - 1),
                            )
                        copy_fn(tmp_slice(c0, nw), t_ps[:, 0:nw])
                    return tmpA, tmpB
                for half, (lo, hi, base) in enumerate(
                        ((0, 2, 0), (2, 5, XSPLIT))):
                    t_ps = pool_fns[half]()
                    for c0, nw, halves in XCH_M1[lo:hi]:
                        for i, xb in enumerate(halves):
                            nc.tensor.matmul(
                                t_ps[:, c0 - base : c0 - base + nw],
                                lhsT=lhs[xb],
                                rhs=wxT[xb][:, c0 : c0 + nw],
                                start=(i == 0),
                                stop=(i == len(halves) - 1),
                            )
                    wtot = (XCH_M1[hi - 1][0] + XCH_M1[hi - 1][1]) - base
                    copy_fn(tmp_slice(base, wtot), t_ps[:, 0:wtot])
                return tmpA, tmpB

            def _tmp_rhs(tmp2, c0, nw):
                tmpA, tmpB = tmp2
                if c0 < XSPLIT:
                    return tmpA[:, c0 : c0 + nw]
                return tmpB[:, c0 - XSPLIT : c0 - XSPLIT + nw]

            dma_tick = [0]

            def emit_ph2_yc(h, tmp2, yc, copy_fn, pool_fns=None,
                            dma_engs=None):
                """pool_fns=None: four 1-bank chunks (mid rows, low PSUM
                pressure).  pool_fns: two 2-bank tiles + fat copies +
                half-row DMAs (tail)."""
                ob = outsb.tile([128, WP], FP16, name="ob", tag="ob")
                if pool_fns is None:
                    for c0, nw in XCH:
                        o_ps = outps.tile([128, 512], F32, name="o_ps",
                                          tag="o_ps")
                        nc.tensor.matmul(
                            o_ps[:, 0:nw],
                            lhsT=wyT[:, yc * 128 : yc * 128 + 128],
                            rhs=_tmp_rhs(tmp2, c0, nw),
                            start=True, stop=True,
                        )
                        copy_fn(ob[:, c0 : c0 + nw], o_ps[:, 0:nw])
                    nc.sync.dma_start(
                        out=out[h, yc * 128 : yc * 128 + 128, :], in_=ob)
                    return
                for half, (cs_, cw) in enumerate(
                        ((0, XSPLIT), (XSPLIT, WP - XSPLIT))):
                    o_ps = pool_fns[half]()
                    for ci in range(2):
                        c0, nw = XCH[half * 2 + ci]
                        nc.tensor.matmul(
                            o_ps[:, c0 - cs_ : c0 - cs_ + nw],
                            lhsT=wyT[:, yc * 128 : yc * 128 + 128],
                            rhs=_tmp_rhs(tmp2, c0, nw),
                            start=True, stop=True,
                        )
                    copy_fn(ob[:, cs_ : cs_ + cw], o_ps[:, 0:cw])
                    if dma_engs is not None:
                        eng = dma_engs[half]
                    else:
                        eng = ENG[DMA_PAT[dma_tick[0] % len(DMA_PAT)]]
                        dma_tick[0] += 1
                    eng.dma_start(
                        out=out[h, yc * 128 : yc * 128 + 128,
                                cs_ : cs_ + cw],
                        in_=ob[:, cs_ : cs_ + cw])

            # sample 0 cost volume + regression; Pool takes POOL_G groups.
            # the last two groups are 2 disparities each so the final
            # [abs -> cs -> exp -> st -> pred] chain is short
            GRP = [(0, 4), (4, 4), (8, 4), (12, 4), (16, 4), (20, 2),
                   (22, 2)]
            NG = len(GRP)
            POOL_G0 = set(cfg["pool_g0"])
            POOL_G1 = set(cfg["pool_g1"])
            for g in range(NG):
                emit_ph1_group(0, *GRP[g],
                               eng=nc.gpsimd if g in POOL_G0 else None,
                               split_exp=(g == NG - 1), last_g=(g == NG - 1))
            emit_ph1_group(1, *GRP[0])
            emit_pred(0)
            tmp0 = emit_ph2_head(0, copy_mid)
            for g in range(1, NG):
                emit_ph1_group(1, *GRP[g],
                               eng=nc.gpsimd if g in POOL_G1 else None,
                               split_exp=(g == NG - 1), last_g=(g == NG - 1))
                emit_ph2_yc(0, tmp0, g - 1, copy_mid)   # rows 0..5
            emit_pred(1)
            ph1_stack.close()  # free cost (4) + s/t (2) banks for the tail
            with tc.tile_pool(name="pstail", bufs=3, space="PSUM") as pstail:
                def tail_tile():
                    return pstail.tile([128, 1024], F32, name="tl",
                                       tag="tl")

                tail_pools = (tail_tile, tail_tile)
                emit_ph2_yc(0, tmp0, 6, copy_tail, tail_pools)
                emit_ph2_yc(0, tmp0, 7, copy_tail, tail_pools)
                tmp1 = emit_ph2_head(1, copy_tail, tail_pools)
                for yc in range(8):
                    emit_ph2_yc(1, tmp1, yc, copy_tail, tail_pools)
    nc.compile()
    return nc


_NC_CACHE = [None]


def kernel(feat_l, feat_r, img_h, img_w):
    feat_l = np.asarray(feat_l, dtype=np.float32)
    feat_r = np.asarray(feat_r, dtype=np.float32)
    assert int(img_h) == HI and int(img_w) == WI
    assert feat_l.shape == (B, C, H0, W0)

    if _NC_CACHE[0] is None:
        _NC_CACHE[0] = _build()
    nc = _NC_CACHE[0]

    consts = _host_consts()
    in_maps = []
    for c in range(NCORES):
        fl = _pack_feat(feat_l[SPC * c : SPC * c + SPC].astype(np.float16))
        fr = _pack_feat_padded(
            feat_r[SPC * c : SPC * c + SPC].astype(np.float16))
        in_maps.append({"lf": fl, "rf": fr, **consts})

    res = run_bass_kernel_spmd(nc, in_maps, core_ids=list(range(NCORES)),
                               trace=_TRACE[0])
    outs = [res.results[i]["out"].astype(np.float32) for i in range(NCORES)]
    full = np.concatenate(outs, axis=0).reshape(B, 1, HI, WI)
    kernel._last_exec_ns = res.exec_time_ns
    return full


# revision 69
# speedup vs baseline: 1.3383x; 1.0125x over previous
"""Stereo cost-volume + softmax disparity regression + bilinear upsample.

Full inputs:  feat_l, feat_r [16, 4, 128, 240] f32, img_h=1024, img_w=1920.
Full output:  [16, 1, 1024, 1920] f32.

Sharding: pure data parallel, 2 samples per core across 8 cores; the two
samples run as a software pipeline (sample 1's cost volume overlaps
sample 0's upsample).

Phase 1 (7 disparity groups per sample: 5x4 + 2x2, the small ones last so
the final [abs -> cs -> exp -> st -> pred] chain is short):
  - DVE computes |L - R(x-d)| for a whole group in ONE subtract (custom
    4D access pattern walking the host-pre-padded feat_r window at
    stride 1) + ONE 4x-mode u16 bitwise abs.  Pool (GPSIMD) runs the
    subtract for a few groups in parallel (it cannot run the bitwise
    abs or touch PSUM -- BIR verifier rules -- so DVE abs's those too).
  - Channel sum runs "flipped" on the PE: the diff chunk [128, 120] is
    the stationary lhsT and the [128, 32] selector streams, producing
    cost chunks [120(x), 32(y32)] -- 4x fewer streamed columns than
    streaming the diff.  Layout: cost[x-block, (sec, yb, xb, y32)].
  - ACT exponentiates a whole group tile [120, <=1024] at once.
  - s/t accumulate in PSUM via scaled-identity lhsT matmuls (s += e,
    t += 8d*e) into [120, 32] regions keyed (xb, yb); PSUM lazy-zero
    semantics allow interleaved region accumulation with start exactly
    once per bank (skip_group_check).
Phase 2: pred = t * (1/s) comes out ALREADY x-transposed [120(x), (xb,
  yb, y32)], so M1 (x-interp) consumes per-xb contiguous [120, 128]
  slices as lhsT -- no PE transposes.  M1/M2/output all split X at 956,
  the exact pure-A/pure-B wxT boundary (only an 8-column sliver needs
  both x-halves), so each M2 row-half depends on one M1 copy only.
  PSUM->SBUF copies go to ACT (and DVE in the tail; GPSIMD cannot read
  PSUM); the tail uses 2-bank PSUM tiles with one fat copy per row-half.
  Output rows DMA on the idle SP queue, plus ACT/Pool queues in the
  tail, split at the 956 boundary to overlap drain with the last rows.

All engine assignments (which groups Pool subtracts, copy-lane patterns,
DMA queue pattern) live in CFG, tuned by sweeping CoreSim.
"""
import sys

sys.path.insert(0, "/opt/trn_rl_repo")

import numpy as np

import concourse.bass as bass
import concourse.bacc as bacc
import concourse.tile as tile
import concourse.mybir as mybir
from concourse.bass_utils import run_bass_kernel_spmd

# ---------------------------------------------------------------- constants
B, C, H0, W0 = 16, 4, 128, 240
D = 24             # disparities
NCORES = 8
SPC = B // NCORES  # samples per core = 2
HI, WI = 1024, 1920
WP = WI
XB = 120           # x-block width (two blocks per row)
XSPLIT = 956       # X column where the x-interp flips from wxT half A to B
# M2 / output X chunks (PSUM <= 512 cols each, split at XSPLIT)
XCH = [(0, 512), (512, 444), (956, 512), (1468, 452)]
# M1 X chunks: (start, width, x-halves needed); grouped so chunks 0-1 and
# 2-4 each pack into one 2-bank PSUM tile without bank-straddling writes
XCH_M1 = [(0, 512, (0,)), (512, 444, (0,)), (956, 8, (0, 1)),
          (964, 504, (1,)), (1468, 452, (1,))]
YB = H0 // 32      # 4 y-blocks
G = SPC * YB       # 8 feat groups (sample-major)
FREE = G * W0      # 1920
PAD = 28           # left-pad columns in padded feat_r groups
GW = W0 + 2 * PAD  # padded group width (even)
EXP_BIAS = 8.0

FP16 = mybir.dt.float16
F32 = mybir.dt.float32
U16 = mybir.dt.uint16

_TRACE = [False]


# ------------------------------------------------------------- host weights
def _host_consts():
    # selector for the flipped channel sum: sel[ch*32+y32, y'] = (y32 == y')
    sel = np.zeros((128, 32), np.float16)
    for ch in range(C):
        sel[ch * 32 : (ch + 1) * 32, :] = np.eye(32, dtype=np.float16)

    # s identity and per-disparity t identities (8*d scaling)
    sid = np.eye(XB, dtype=np.float16)
    tid = np.zeros((XB, D * XB), np.float16)
    for d in range(D):
        tid[:, d * XB : (d + 1) * XB] = np.eye(XB, dtype=np.float16) * \
            np.float16(8.0 * d)

    # x-interp weights wxT[x, X], f32 linspace to match jnp rounding
    xs = np.linspace(0.0, W0 - 1.0, WI, dtype=np.float32)
    x0 = np.floor(xs).astype(np.int64)
    x1 = np.minimum(x0 + 1, W0 - 1)
    wx = (xs - x0).astype(np.float32)
    wxT_full = np.zeros((W0, WI), np.float32)
    wxT_full[x0, np.arange(WI)] += 1.0 - wx
    wxT_full[x1, np.arange(WI)] += wx
    # chunk validity: columns left of 956 only use x<120; right of 964 only
    # x>=120; the 8-col sliver uses both
    assert x1[:956].max() <= XB - 1
    assert x0[964:].min() >= XB
    wxTa = wxT_full[0:XB]
    wxTb = wxT_full[XB : 2 * XB]

    # y-interp weights wyT[y, Y]
    ys = np.linspace(0.0, H0 - 1.0, HI, dtype=np.float32)
    y0 = np.floor(ys).astype(np.int64)
    y1 = np.minimum(y0 + 1, H0 - 1)
    wy = (ys - y0).astype(np.float32)
    wyT = np.zeros((H0, HI), np.float32)
    wyT[y0, np.arange(HI)] += 1.0 - wy
    wyT[y1, np.arange(HI)] += wy

    return {
        "sel": sel,
        "sid": sid,
        "tid": tid,
        "wxTa": wxTa.astype(np.float16),
        "wxTb": wxTb.astype(np.float16),
        "wyT": wyT.astype(np.float16),
    }


def _pack_feat(f):
    """[SPC, C, H0, W0] -> [128, FREE] with p=(ch,y32), free=(s,yb,x)."""
    a = f.reshape(SPC, C, YB, 32, W0)
    a = np.ascontiguousarray(a.transpose(1, 3, 0, 2, 4))  # ch,y32,s,yb,x
    return a.reshape(128, FREE)


def _pack_feat_padded(f):
    """[SPC, C, H0, W0] -> [128, SPC*YB*GW], PAD zero cols around each row."""
    a = f.reshape(SPC, C, YB, 32, W0).transpose(1, 3, 0, 2, 4)
    p = np.zeros((C, 32, SPC, YB, GW), f.dtype)
    p[:, :, :, :, PAD : PAD + W0] = a
    return p.reshape(128, SPC * YB * GW)


# scheduling configuration (engine assignment knobs, tuned via sweep)
CFG = {
    "pool_g0": (1, 2, 3),
    "pool_g1": (1, 2, 3),
    "mid_pat": "AAV",
    "tail_pat": "VA",
    "dma_pat": "PS",    # tail out-DMA queues: S=SP, A=ACT, P=Pool
    "mult1": "V",         # engine for sample-1 pred multiply
}


# ------------------------------------------------------------- build kernel
def _build(cfg=None):
    cfg = {**CFG, **(cfg or {})}
    nc = bacc.Bacc("TRN2", target_bir_lowering=False, debug=False,
                   num_devices=NCORES)
    lf = nc.dram_tensor("lf", [128, FREE], FP16, kind="ExternalInput").ap()
    rf = nc.dram_tensor("rf", [128, SPC * YB * GW], FP16,
                        kind="ExternalInput").ap()
    sel_d = nc.dram_tensor("sel", [128, 32], FP16, kind="ExternalInput").ap()
    sid_d = nc.dram_tensor("sid", [XB, XB], FP16, kind="ExternalInput").ap()
    tid_d = nc.dram_tensor("tid", [XB, D * XB], FP16,
                           kind="ExternalInput").ap()
    wxa_d = nc.dram_tensor("wxTa", [XB, WI], FP16, kind="ExternalInput").ap()
    wxb_d = nc.dram_tensor("wxTb", [XB, WI], FP16, kind="ExternalInput").ap()
    wyT_d = nc.dram_tensor("wyT", [H0, HI], FP16, kind="ExternalInput").ap()
    out = nc.dram_tensor("out", [SPC, HI, WI], FP16,
                         kind="ExternalOutput").ap()

    AF = mybir.ActivationFunctionType
    OP = mybir.AluOpType

    with tile.TileContext(nc) as tc:
        with (
            tc.tile_pool(name="consts", bufs=1) as consts,
            tc.tile_pool(name="feat", bufs=1) as feat,
            tc.tile_pool(name="diff", bufs=6) as diffp,
            tc.tile_pool(name="ep", bufs=6) as ep,
            tc.tile_pool(name="predp", bufs=1) as predp,
            tc.tile_pool(name="upsb", bufs=1) as upsb,
            tc.tile_pool(name="outsb", bufs=8) as outsb,
            tc.tile_pool(name="outps", bufs=2, space="PSUM") as outps,
        ):
            from contextlib import ExitStack
            ph1_stack = ExitStack()
            costp = ph1_stack.enter_context(
                tc.tile_pool(name="costp", bufs=2, space="PSUM"))
            stps = ph1_stack.enter_context(
                tc.tile_pool(name="stps", bufs=1, space="PSUM"))

            bias8 = consts.tile([XB, 1], F32)
            nc.vector.memset(bias8, EXP_BIAS)

            # ---- features: left halves on SP; padded right h0 on Pool
            # (ahead of ACT's table load), h1 on ACT.
            # per-sample L tiles so sample-0's subs don't wait on h1's DMA
            Lh = []
            for h2 in range(SPC):
                Lt = feat.tile([128, FREE // 2], FP16, tag=f"L{h2}",
                               name=f"L{h2}")
                nc.sync.dma_start(
                    out=Lt,
                    in_=lf[:, h2 * (FREE // 2) : (h2 + 1) * (FREE // 2)])
                Lh.append(Lt.rearrange("p (g w) -> p g w", w=W0))
            R = [None, None]  # R[h] -> [128, YB, GW]
            for h2 in range(SPC):
                Rt = feat.tile([128, YB * GW], FP16,
                               tag=f"rpad{h2}", name=f"rpad{h2}")
                dma_eng = nc.gpsimd if h2 == 0 else nc.scalar
                dma_eng.dma_start(
                    out=Rt,
                    in_=rf[:, h2 * YB * GW : (h2 + 1) * YB * GW])
                R[h2] = Rt.rearrange("p (g w) -> p g w", w=GW)

            # ---- constants on the SP queue (after features)
            sel = consts.tile([128, 32], FP16)
            nc.sync.dma_start(out=sel, in_=sel_d)
            sid = consts.tile([XB, XB], FP16)
            nc.sync.dma_start(out=sid, in_=sid_d)
            tid = consts.tile([XB, D * XB], FP16)
            nc.sync.dma_start(out=tid, in_=tid_d)
            wxT = [consts.tile([XB, WI], FP16, name=f"wxT{i}", tag=f"wxT{i}")
                   for i in range(2)]
            nc.sync.dma_start(out=wxT[0], in_=wxa_d)
            nc.sync.dma_start(out=wxT[1], in_=wxb_d)
            wyT = consts.tile([128, HI], FP16)
            nc.sync.dma_start(out=wyT, in_=wyT_d)

            st = [stps.tile([XB, 512], F32, name=f"st{h}", tag=f"st{h}")
                  for h in range(SPC)]

            # ---------- copy lanes
            mid_tick = [0]
            tail_tick = [0]

            def _copy_on(eng, dst, src):
                # PSUM->SBUF moves: ACT or DVE only (GPSIMD cannot touch
                # PSUM -- the BIR verifier rejects it)
                if eng == "A":
                    nc.scalar.copy(out=dst, in_=src)
                else:
                    nc.vector.tensor_copy(out=dst, in_=src)

            MID_PAT = list(cfg["mid_pat"])
            TAIL_PAT = list(cfg["tail_pat"])
            DMA_PAT = list(cfg["dma_pat"])
            ENG = {"S": nc.sync, "A": nc.scalar, "P": nc.gpsimd,
                   "V": nc.vector}

            def copy_mid(dst, src):
                _copy_on(MID_PAT[mid_tick[0] % len(MID_PAT)], dst, src)
                mid_tick[0] += 1

            def copy_tail(dst, src):
                _copy_on(TAIL_PAT[tail_tick[0] % len(TAIL_PAT)], dst, src)
                tail_tick[0] += 1

            # ============ software pipeline over the two samples =========
            pred = [None, None]
            # diff tile sections hold disparities hi-first: [d+3,d+2,d+1,d]
            st_open = [False, False]

            def emit_ph1_group(h, d0, nsec, eng=None, split_exp=False,
                               last_g=False):
                eng = eng or nc.vector
                Dt = diffp.tile([128, 4 * YB * W0], FP16, name="diff",
                                tag="diff")
                D4 = Dt.rearrange("p (s g w) -> p s g w", s=4, w=W0)[:, 0:nsec]
                Lk = Lh[h].unsqueeze(1).broadcast_to([128, nsec, YB, W0])
                # one subtract covers disparities d0+nsec-1..d0 via a k-dim
                # stepping the feat_r window right by 1
                off_hi = PAD - (d0 + nsec - 1)
                Rbase = R[h][:, :, off_hi : off_hi + W0]
                Rk = bass.AP(
                    Rbase.tensor, Rbase.offset,
                    [list(Rbase.ap[0]), [1, nsec],
                     list(Rbase.ap[1]), list(Rbase.ap[2])])
                eng.tensor_tensor(out=D4, in0=Lk, in1=Rk, op=OP.subtract)
                Du = Dt.bitcast(U16)[:, 0 : nsec * YB * W0]
                nc.vector.tensor_scalar(
                    out=Du, in0=Du, scalar1=0x7FFF, scalar2=None,
                    op0=OP.bitwise_and,
                )
                D3 = Dt.rearrange("p (s f) -> p s f", s=4)
                # flipped channel sum: cost[x, (sec, yb, xb, y32)]
                cost = costp.tile([XB, 1024], F32, name="cost", tag="cost")
                e = ep.tile([XB, 1024], FP16, name="e", tag="e")
                for sec in range(nsec):
                    for yb in range(YB):
                        for xb in range(2):
                            nc.tensor.matmul(
                                cost[0:XB,
                                     sec * 256 + yb * 64 + xb * 32 :
                                     sec * 256 + yb * 64 + xb * 32 + 32],
                                lhsT=D3[:, sec,
                                        yb * W0 + xb * XB :
                                        yb * W0 + xb * XB + XB],
                                rhs=sel,
                                start=(sec in (0, 2) and yb == 0 and xb == 0),
                                stop=(sec in (1, 3, nsec - 1)
                                      and yb == YB - 1 and xb == 1),
                                skip_group_check=True,
                            )
                ncols = nsec * 256
                if split_exp and nsec > 2:
                    # half-exps emitted back-to-back so the tail chain
                    # [cs -> exp -> st -> pred] is as short as possible
                    for hf in range(2):
                        nc.scalar.activation(
                            out=e[:, hf * ncols // 2 : ncols // 2 * (hf + 1)],
                            in_=cost[:, hf * ncols // 2 :
                                     ncols // 2 * (hf + 1)],
                            func=AF.Exp, bias=bias8, scale=-1.0)
                else:
                    nc.scalar.activation(out=e[:, 0:ncols],
                                         in_=cost[:, 0:ncols],
                                         func=AF.Exp, bias=bias8, scale=-1.0)
                for sec in range(nsec):
                    _emit_st_sec(h, d0 + (nsec - 1 - sec), sec, e, last_g
                                 and sec == nsec - 1)

            def _emit_st_sec(h, d, sec, e, last_sec):
                for yb in range(YB):
                    for xb in range(2):
                        ecol = sec * 256 + yb * 64 + xb * 32
                        scol = xb * 256 + yb * 64
                        first = not st_open[h]
                        st_open[h] = True
                        last = last_sec and yb == YB - 1 and xb == 1
                        rhs = e[:, ecol : ecol + 32]
                        nc.tensor.matmul(
                            st[h][0:XB, scol : scol + 32],
                            lhsT=sid, rhs=rhs,
                            start=first, stop=False,
                            skip_group_check=True,
                        )
                        nc.tensor.matmul(
                            st[h][0:XB, scol + 32 : scol + 64],
                            lhsT=tid[:, d * XB : d * XB + XB], rhs=rhs,
                            start=False, stop=last,
                            skip_group_check=True,
                        )

            def _strips(ap2d, off):
                # [120, 512] -> [120, 8, 32] strips at (yb, xb) stride 64
                return bass.AP(ap2d.tensor, ap2d.offset + off,
                               [list(ap2d.ap[0]), [64, 8], [1, 32]])

            def emit_pred(h):
                pr = predp.tile([XB, 256], FP16, name=f"pred{h}",
                                tag=f"pred{h}")
                rs = predp.tile([XB, 256], F32, name=f"rs{h}", tag=f"rs{h}")
                nc.vector.reciprocal(out=rs, in_=_strips(st[h], 0))
                nc.vector.tensor_tensor(out=pr, in0=_strips(st[h], 32),
                                        in1=rs, op=OP.mult)
                pred[h] = pr

            def emit_ph2_head(h, copy_fn, pool_fns=None):
                """M1 -> (tmpA, tmpB); fat tiles when pool_fns given."""
                pr = pred[h]
                lhs = [pr[:, xb * 128 : xb * 128 + 128]
                       for xb in range(2)]
                tmpA = upsb.tile([128, XSPLIT], FP16, tag=f"tmpA{h}",
                                 name=f"tmpA{h}")
                tmpB = upsb.tile([128, WP - XSPLIT], FP16, tag=f"tmpB{h}",
                                 name=f"tmpB{h}")

                def tmp_slice(base, wtot):
                    if base < XSPLIT:
                        return tmpA[:, base : base + wtot]
                    return tmpB[:, base - XSPLIT : base - XSPLIT + wtot]

                if pool_fns is None:
                    for c0, nw, halves in XCH_M1:
                        t_ps = outps.tile([128, 512], F32, name="o_ps",
                                          tag="o_ps")
                        for i, xb in enumerate(halves):
                            nc.tensor.matmul(
                                t_ps[:, 0:nw], lhsT=lhs[xb],
                                rhs=wxT[xb][:, c0 : c0 + nw],
                                start=(i == 0), stop=(i == len(halves) - 1),
                            )
                        copy_fn(tmp_slice(c0, nw), t_ps[:, 0:nw])
                    return tmpA, tmpB
                for half, (lo, hi, base) in enumerate(
                        ((0, 2, 0), (2, 5, XSPLIT))):
                    t_ps = pool_fns[half]()
                    for c0, nw, halves in XCH_M1[lo:hi]:
                        for i, xb in enumerate(halves):
                            nc.tensor.matmul(
                                t_ps[:, c0 - base : c0 - base + nw],
                                lhsT=lhs[xb],
                                rhs=wxT[xb][:, c0 : c0 + nw],
                                start=(i == 0),
                                stop=(i == len(halves) - 1),
                            )
                    wtot = (XCH_M1[hi - 1][0] + XCH_M1[hi - 1][1]) - base
                    copy_fn(tmp_slice(base, wtot), t_ps[:, 0:wtot])
                return tmpA, tmpB

            def _tmp_rhs(tmp2, c0, nw):
                tmpA, tmpB = tmp2
                if c0 < XSPLIT:
                    return tmpA[:, c0 : c0 + nw]
                return tmpB[:, c0 - XSPLIT : c0 - XSPLIT + nw]

            dma_tick = [0]

            def emit_ph2_yc(h, tmp2, yc, copy_fn, pool_fns=None,
                            dma_engs=None):
                """pool_fns=None: four 1-bank chunks (mid rows, low PSUM
                pressure).  pool_fns: two 2-bank tiles + fat copies +
                half-row DMAs (tail)."""
                ob = outsb.tile([128, WP], FP16, name="ob", tag="ob")
                if pool_fns is None:
                    for c0, nw in XCH:
                        o_ps = outps.tile([128, 512], F32, name="o_ps",
                                          tag="o_ps")
                        nc.tensor.matmul(
                            o_ps[:, 0:nw],
                            lhsT=wyT[:, yc * 128 : yc * 128 + 128],
                            rhs=_tmp_rhs(tmp2, c0, nw),
                            start=True, stop=True,
                        )
                        copy_fn(ob[:, c0 : c0 + nw], o_ps[:, 0:nw])
                    nc.sync.dma_start(
                        out=out[h, yc * 128 : yc * 128 + 128, :], in_=ob)
                    return
                for half, (cs_, cw) in enumerate(
                        ((0, XSPLIT), (XSPLIT, WP - XSPLIT))):
                    o_ps = pool_fns[half]()
                    for ci in range(2):
                        c0, nw = XCH[half * 2 + ci]
                        nc.tensor.matmul(
                            o_ps[:, c0 - cs_ : c0 - cs_ + nw],
                            lhsT=wyT[:, yc * 128 : yc * 128 + 128],
                            rhs=_tmp_rhs(tmp2, c0, nw),
                            start=True, stop=True,
                        )
                    copy_fn(ob[:, cs_ : cs_ + cw], o_ps[:, 0:cw])
                    if dma_engs is not None:
                        eng = dma_engs[half]
                    else:
                        eng = ENG[DMA_PAT[dma_tick[0] % len(DMA_PAT)]]
                        dma_tick[0] += 1
                    eng.dma_start(
                        out=out[h, yc * 128 : yc * 128 + 128,
                                cs_ : cs_ + cw],
                        in_=ob[:, cs_ : cs_ + cw])

            # sample 0 cost volume + regression; Pool takes POOL_G groups.
            # the last two groups are 2 disparities each so the final
            # [abs -> cs -> exp -> st -> pred] chain is short
            GRP = [(0, 4), (4, 4), (8, 4), (12, 4), (16, 4), (20, 2),
                   (22, 2)]
            NG = len(GRP)
            POOL_G0 = set(cfg["pool_g0"])
            POOL_G1 = set(cfg["pool_g1"])
            for g in range(NG):
                emit_ph1_group(0, *GRP[g],
                               eng=nc.gpsimd if g in POOL_G0 else None,
                               split_exp=(g == NG - 1), last_g=(g == NG - 1))
            emit_ph1_group(1, *GRP[0])
            emit_pred(0)
            tmp0 = emit_ph2_head(0, copy_mid)
            for g in range(1, NG):
                emit_ph1_group(1, *GRP[g],
                               eng=nc.gpsimd if g in POOL_G1 else None,
                               split_exp=(g == NG - 1), last_g=(g == NG - 1))
                emit_ph2_yc(0, tmp0, g - 1, copy_mid)   # rows 0..5
            emit_pred(1)
            ph1_stack.close()  # free cost (4) + s/t (2) banks for the tail
            with tc.tile_pool(name="pstail", bufs=3, space="PSUM") as pstail:
                def tail_tile():
                    return pstail.tile([128, 1024], F32, name="tl",
                                       tag="tl")

                tail_pools = (tail_tile, tail_tile)
                emit_ph2_yc(0, tmp0, 6, copy_tail, tail_pools)
                emit_ph2_yc(0, tmp0, 7, copy_tail, tail_pools)
                tmp1 = emit_ph2_head(1, copy_tail, tail_pools)
                for yc in range(8):
                    emit_ph2_yc(1, tmp1, yc, copy_tail, tail_pools)
    nc.compile()
    return nc


_NC_CACHE = [None]


def kernel(feat_l, feat_r, img_h, img_w):
    feat_l = np.asarray(feat_l, dtype=np.float32)
    feat_r = np.asarray(feat_r, dtype=np.float32)
    assert int(img_h) == HI and int(img_w) == WI
    assert feat_l.shape == (B, C, H0, W0)

    if _NC_CACHE[0] is None:
        _NC_CACHE[0] = _build()
    nc = _NC_CACHE[0]

    consts = _host_consts()
    in_maps = []
    for c in range(NCORES):
        fl = _pack_feat(feat_l[SPC * c : SPC * c + SPC].astype(np.float16))
        fr = _pack_feat_padded(
            feat_r[SPC * c : SPC * c + SPC].astype(np.float16))
        in_maps.append({"lf": fl, "rf": fr, **consts})

    res = run_bass_kernel_spmd(nc, in_maps, core_ids=list(range(NCORES)),
                               trace=_TRACE[0])
    outs = [res.results[i]["out"].astype(np.float32) for i in range(NCORES)]
    full = np.concatenate(outs, axis=0).reshape(B, 1, HI, WI)
    kernel._last_exec_ns = res.exec_time_ns
    return full


# revision 70
# speedup vs baseline: 1.3551x; 1.0125x over previous
"""Stereo cost-volume + softmax disparity regression + bilinear upsample.

Full inputs:  feat_l, feat_r [16, 4, 128, 240] f32, img_h=1024, img_w=1920.
Full output:  [16, 1, 1024, 1920] f32.

Sharding: pure data parallel, 2 samples per core across 8 cores; the two
samples run as a software pipeline (sample 1's cost volume overlaps
sample 0's upsample).

Phase 1 (7 disparity groups per sample: 5x4 + 2x2, the small ones last so
the final [abs -> cs -> exp -> st -> pred] chain is short):
  - DVE computes |L - R(x-d)| for a whole group in ONE subtract (custom
    4D access pattern walking the host-pre-padded feat_r window at
    stride 1) + ONE 4x-mode u16 bitwise abs.  Pool (GPSIMD) runs the
    subtract for a few groups in parallel (it cannot run the bitwise
    abs or touch PSUM -- BIR verifier rules -- so DVE abs's those too).
  - Channel sum runs "flipped" on the PE: the diff chunk [128, 120] is
    the stationary lhsT and the [128, 32] selector streams, producing
    cost chunks [120(x), 32(y32)] -- 4x fewer streamed columns than
    streaming the diff.  Layout: cost[x-block, (sec, yb, xb, y32)].
  - ACT exponentiates a whole group tile [120, <=1024] at once.
  - s/t accumulate in PSUM via scaled-identity lhsT matmuls (s += e,
    t += 8d*e) into [120, 32] regions keyed (xb, yb); PSUM lazy-zero
    semantics allow interleaved region accumulation with start exactly
    once per bank (skip_group_check).
Phase 2: pred = t * (1/s) comes out ALREADY x-transposed [120(x), (xb,
  yb, y32)], so M1 (x-interp) consumes per-xb contiguous [120, 128]
  slices as lhsT -- no PE transposes.  M1/M2/output all split X at 956,
  the exact pure-A/pure-B wxT boundary (only an 8-column sliver needs
  both x-halves), so each M2 row-half depends on one M1 copy only.
  PSUM->SBUF copies go to ACT (and DVE in the tail; GPSIMD cannot read
  PSUM); the tail uses 2-bank PSUM tiles with one fat copy per row-half.
  Output rows DMA on the idle SP queue, plus ACT/Pool queues in the
  tail, split at the 956 boundary to overlap drain with the last rows.

All engine assignments (which groups Pool subtracts, copy-lane patterns,
DMA queue pattern) live in CFG, tuned by sweeping CoreSim.
"""
import sys

sys.path.insert(0, "/opt/trn_rl_repo")

import numpy as np

import concourse.bass as bass
import concourse.bacc as bacc
import concourse.tile as tile
import concourse.mybir as mybir
from concourse.bass_utils import run_bass_kernel_spmd

# ---------------------------------------------------------------- constants
B, C, H0, W0 = 16, 4, 128, 240
D = 24             # disparities
NCORES = 8
SPC = B // NCORES  # samples per core = 2
HI, WI = 1024, 1920
WP = WI
XB = 120           # x-block width (two blocks per row)
XSPLIT = 956       # X column where the x-interp flips from wxT half A to B
# M2 / output X chunks (PSUM <= 512 cols each, split at XSPLIT)
XCH = [(0, 512), (512, 444), (956, 512), (1468, 452)]
# M1 X chunks: (start, width, x-halves needed); grouped so chunks 0-1 and
# 2-4 each pack into one 2-bank PSUM tile without bank-straddling writes
XCH_M1 = [(0, 512, (0,)), (512, 444, (0,)), (956, 8, (0, 1)),
          (964, 504, (1,)), (1468, 452, (1,))]
YB = H0 // 32      # 4 y-blocks
G = SPC * YB       # 8 feat groups (sample-major)
FREE = G * W0      # 1920
PAD = 28           # left-pad columns in padded feat_r groups
GW = W0 + 2 * PAD  # padded group width (even)
EXP_BIAS = 8.0

FP16 = mybir.dt.float16
F32 = mybir.dt.float32
U16 = mybir.dt.uint16

_TRACE = [False]


# ------------------------------------------------------------- host weights
def _host_consts():
    # selector for the flipped channel sum: sel[ch*32+y32, y'] = (y32 == y')
    sel = np.zeros((128, 32), np.float16)
    for ch in range(C):
        sel[ch * 32 : (ch + 1) * 32, :] = np.eye(32, dtype=np.float16)

    # s identity and per-disparity t identities (8*d scaling)
    sid = np.eye(XB, dtype=np.float16)
    tid = np.zeros((XB, D * XB), np.float16)
    for d in range(D):
        tid[:, d * XB : (d + 1) * XB] = np.eye(XB, dtype=np.float16) * \
            np.float16(8.0 * d)

    # x-interp weights wxT[x, X], f32 linspace to match jnp rounding
    xs = np.linspace(0.0, W0 - 1.0, WI, dtype=np.float32)
    x0 = np.floor(xs).astype(np.int64)
    x1 = np.minimum(x0 + 1, W0 - 1)
    wx = (xs - x0).astype(np.float32)
    wxT_full = np.zeros((W0, WI), np.float32)
    wxT_full[x0, np.arange(WI)] += 1.0 - wx
    wxT_full[x1, np.arange(WI)] += wx
    # chunk validity: columns left of 956 only use x<120; right of 964 only
    # x>=120; the 8-col sliver uses both
    assert x1[:956].max() <= XB - 1
    assert x0[964:].min() >= XB
    wxTa = wxT_full[0:XB]
    wxTb = wxT_full[XB : 2 * XB]

    # y-interp weights wyT[y, Y]
    ys = np.linspace(0.0, H0 - 1.0, HI, dtype=np.float32)
    y0 = np.floor(ys).astype(np.int64)
    y1 = np.minimum(y0 + 1, H0 - 1)
    wy = (ys - y0).astype(np.float32)
    wyT = np.zeros((H0, HI), np.float32)
    wyT[y0, np.arange(HI)] += 1.0 - wy
    wyT[y1, np.arange(HI)] += wy

    return {
        "sel": sel,
        "sid": sid,
        "tid": tid,
        "wxTa": wxTa.astype(np.float16),
        "wxTb": wxTb.astype(np.float16),
        "wyT": wyT.astype(np.float16),
    }


def _pack_feat(f):
    """[SPC, C, H0, W0] -> [128, FREE] with p=(ch,y32), free=(s,yb,x)."""
    a = f.reshape(SPC, C, YB, 32, W0)
    a = np.ascontiguousarray(a.transpose(1, 3, 0, 2, 4))  # ch,y32,s,yb,x
    return a.reshape(128, FREE)


def _pack_feat_padded(f):
    """[SPC, C, H0, W0] -> [128, SPC*YB*GW], PAD zero cols around each row."""
    a = f.reshape(SPC, C, YB, 32, W0).transpose(1, 3, 0, 2, 4)
    p = np.zeros((C, 32, SPC, YB, GW), f.dtype)
    p[:, :, :, :, PAD : PAD + W0] = a
    return p.reshape(128, SPC * YB * GW)


# scheduling configuration (engine assignment knobs, tuned via sweep)
CFG = {
    "pool_g0": (1, 2, 3),
    "pool_g1": (1, 2, 4),
    "mid_pat": "AAV",
    "tail_pat": "VA",
    "dma_pat": "PS",    # tail out-DMA queues: S=SP, A=ACT, P=Pool
    "mult1": "V",         # engine for sample-1 pred multiply
}


# ------------------------------------------------------------- build kernel
def _build(cfg=None):
    cfg = {**CFG, **(cfg or {})}
    nc = bacc.Bacc("TRN2", target_bir_lowering=False, debug=False,
                   num_devices=NCORES)
    lf = nc.dram_tensor("lf", [128, FREE], FP16, kind="ExternalInput").ap()
    rf = nc.dram_tensor("rf", [128, SPC * YB * GW], FP16,
                        kind="ExternalInput").ap()
    sel_d = nc.dram_tensor("sel", [128, 32], FP16, kind="ExternalInput").ap()
    sid_d = nc.dram_tensor("sid", [XB, XB], FP16, kind="ExternalInput").ap()
    tid_d = nc.dram_tensor("tid", [XB, D * XB], FP16,
                           kind="ExternalInput").ap()
    wxa_d = nc.dram_tensor("wxTa", [XB, WI], FP16, kind="ExternalInput").ap()
    wxb_d = nc.dram_tensor("wxTb", [XB, WI], FP16, kind="ExternalInput").ap()
    wyT_d = nc.dram_tensor("wyT", [H0, HI], FP16, kind="ExternalInput").ap()
    out = nc.dram_tensor("out", [SPC, HI, WI], FP16,
                         kind="ExternalOutput").ap()

    AF = mybir.ActivationFunctionType
    OP = mybir.AluOpType

    with tile.TileContext(nc) as tc:
        with (
            tc.tile_pool(name="consts", bufs=1) as consts,
            tc.tile_pool(name="feat", bufs=1) as feat,
            tc.tile_pool(name="diff", bufs=6) as diffp,
            tc.tile_pool(name="ep", bufs=6) as ep,
            tc.tile_pool(name="predp", bufs=1) as predp,
            tc.tile_pool(name="upsb", bufs=1) as upsb,
            tc.tile_pool(name="outsb", bufs=8) as outsb,
            tc.tile_pool(name="outps", bufs=2, space="PSUM") as outps,
        ):
            from contextlib import ExitStack
            ph1_stack = ExitStack()
            costp = ph1_stack.enter_context(
                tc.tile_pool(name="costp", bufs=2, space="PSUM"))
            stps = ph1_stack.enter_context(
                tc.tile_pool(name="stps", bufs=1, space="PSUM"))

            bias8 = consts.tile([XB, 1], F32)
            nc.vector.memset(bias8, EXP_BIAS)

            # ---- features: left halves on SP; padded right h0 on Pool
            # (ahead of ACT's table load), h1 on ACT.
            # per-sample L tiles so sample-0's subs don't wait on h1's DMA
            Lh = []
            for h2 in range(SPC):
                Lt = feat.tile([128, FREE // 2], FP16, tag=f"L{h2}",
                               name=f"L{h2}")
                nc.sync.dma_start(
                    out=Lt,
                    in_=lf[:, h2 * (FREE // 2) : (h2 + 1) * (FREE // 2)])
                Lh.append(Lt.rearrange("p (g w) -> p g w", w=W0))
            R = [None, None]  # R[h] -> [128, YB, GW]
            for h2 in range(SPC):
                Rt = feat.tile([128, YB * GW], FP16,
                               tag=f"rpad{h2}", name=f"rpad{h2}")
                dma_eng = nc.gpsimd if h2 == 0 else nc.scalar
                dma_eng.dma_start(
                    out=Rt,
                    in_=rf[:, h2 * YB * GW : (h2 + 1) * YB * GW])
                R[h2] = Rt.rearrange("p (g w) -> p g w", w=GW)

            # ---- constants on the SP queue (after features)
            sel = consts.tile([128, 32], FP16)
            nc.sync.dma_start(out=sel, in_=sel_d)
            sid = consts.tile([XB, XB], FP16)
            nc.sync.dma_start(out=sid, in_=sid_d)
            tid = consts.tile([XB, D * XB], FP16)
            nc.sync.dma_start(out=tid, in_=tid_d)
            wxT = [consts.tile([XB, WI], FP16, name=f"wxT{i}", tag=f"wxT{i}")
                   for i in range(2)]
            nc.sync.dma_start(out=wxT[0], in_=wxa_d)
            nc.sync.dma_start(out=wxT[1], in_=wxb_d)
            wyT = consts.tile([128, HI], FP16)
            nc.sync.dma_start(out=wyT, in_=wyT_d)

            st = [stps.tile([XB, 512], F32, name=f"st{h}", tag=f"st{h}")
                  for h in range(SPC)]

            # ---------- copy lanes
            mid_tick = [0]
            tail_tick = [0]

            def _copy_on(eng, dst, src):
                # PSUM->SBUF moves: ACT or DVE only (GPSIMD cannot touch
                # PSUM -- the BIR verifier rejects it)
                if eng == "A":
                    nc.scalar.copy(out=dst, in_=src)
                else:
                    nc.vector.tensor_copy(out=dst, in_=src)

            MID_PAT = list(cfg["mid_pat"])
            TAIL_PAT = list(cfg["tail_pat"])
            DMA_PAT = list(cfg["dma_pat"])
            ENG = {"S": nc.sync, "A": nc.scalar, "P": nc.gpsimd,
                   "V": nc.vector}

            def copy_mid(dst, src):
                _copy_on(MID_PAT[mid_tick[0] % len(MID_PAT)], dst, src)
                mid_tick[0] += 1

            def copy_tail(dst, src):
                _copy_on(TAIL_PAT[tail_tick[0] % len(TAIL_PAT)], dst, src)
                tail_tick[0] += 1

            # ============ software pipeline over the two samples =========
            pred = [None, None]
            # diff tile sections hold disparities hi-first: [d+3,d+2,d+1,d]
            st_open = [False, False]

            def emit_ph1_group(h, d0, nsec, eng=None, split_exp=False,
                               last_g=False):
                eng = eng or nc.vector
                Dt = diffp.tile([128, 4 * YB * W0], FP16, name="diff",
                                tag="diff")
                D4 = Dt.rearrange("p (s g w) -> p s g w", s=4, w=W0)[:, 0:nsec]
                Lk = Lh[h].unsqueeze(1).broadcast_to([128, nsec, YB, W0])
                # one subtract covers disparities d0+nsec-1..d0 via a k-dim
                # stepping the feat_r window right by 1
                off_hi = PAD - (d0 + nsec - 1)
                Rbase = R[h][:, :, off_hi : off_hi + W0]
                Rk = bass.AP(
                    Rbase.tensor, Rbase.offset,
                    [list(Rbase.ap[0]), [1, nsec],
                     list(Rbase.ap[1]), list(Rbase.ap[2])])
                eng.tensor_tensor(out=D4, in0=Lk, in1=Rk, op=OP.subtract)
                Du = Dt.bitcast(U16)[:, 0 : nsec * YB * W0]
                nc.vector.tensor_scalar(
                    out=Du, in0=Du, scalar1=0x7FFF, scalar2=None,
                    op0=OP.bitwise_and,
                )
                D3 = Dt.rearrange("p (s f) -> p s f", s=4)
                # flipped channel sum: cost[x, (sec, yb, xb, y32)]
                cost = costp.tile([XB, 1024], F32, name="cost", tag="cost")
                e = ep.tile([XB, 1024], FP16, name="e", tag="e")
                for sec in range(nsec):
                    for yb in range(YB):
                        for xb in range(2):
                            nc.tensor.matmul(
                                cost[0:XB,
                                     sec * 256 + yb * 64 + xb * 32 :
                                     sec * 256 + yb * 64 + xb * 32 + 32],
                                lhsT=D3[:, sec,
                                        yb * W0 + xb * XB :
                                        yb * W0 + xb * XB + XB],
                                rhs=sel,
                                start=(sec in (0, 2) and yb == 0 and xb == 0),
                                stop=(sec in (1, 3, nsec - 1)
                                      and yb == YB - 1 and xb == 1),
                                skip_group_check=True,
                            )
                ncols = nsec * 256
                if split_exp and nsec > 2:
                    # half-exps emitted back-to-back so the tail chain
                    # [cs -> exp -> st -> pred] is as short as possible
                    for hf in range(2):
                        nc.scalar.activation(
                            out=e[:, hf * ncols // 2 : ncols // 2 * (hf + 1)],
                            in_=cost[:, hf * ncols // 2 :
                                     ncols // 2 * (hf + 1)],
                            func=AF.Exp, bias=bias8, scale=-1.0)
                else:
                    nc.scalar.activation(out=e[:, 0:ncols],
                                         in_=cost[:, 0:ncols],
                                         func=AF.Exp, bias=bias8, scale=-1.0)
                for sec in range(nsec):
                    _emit_st_sec(h, d0 + (nsec - 1 - sec), sec, e, last_g
                                 and sec == nsec - 1)

            def _emit_st_sec(h, d, sec, e, last_sec):
                for yb in range(YB):
                    for xb in range(2):
                        ecol = sec * 256 + yb * 64 + xb * 32
                        scol = xb * 256 + yb * 64
                        first = not st_open[h]
                        st_open[h] = True
                        last = last_sec and yb == YB - 1 and xb == 1
                        rhs = e[:, ecol : ecol + 32]
                        nc.tensor.matmul(
                            st[h][0:XB, scol : scol + 32],
                            lhsT=sid, rhs=rhs,
                            start=first, stop=False,
                            skip_group_check=True,
                        )
                        nc.tensor.matmul(
                            st[h][0:XB, scol + 32 : scol + 64],
                            lhsT=tid[:, d * XB : d * XB + XB], rhs=rhs,
                            start=False, stop=last,
                            skip_group_check=True,
                        )

            def _strips(ap2d, off):
                # [120, 512] -> [120, 8, 32] strips at (yb, xb) stride 64
                return bass.AP(ap2d.tensor, ap2d.offset + off,
                               [list(ap2d.ap[0]), [64, 8], [1, 32]])

            def emit_pred(h):
                pr = predp.tile([XB, 256], FP16, name=f"pred{h}",
                                tag=f"pred{h}")
                rs = predp.tile([XB, 256], F32, name=f"rs{h}", tag=f"rs{h}")
                nc.vector.reciprocal(out=rs, in_=_strips(st[h], 0))
                nc.vector.tensor_tensor(out=pr, in0=_strips(st[h], 32),
                                        in1=rs, op=OP.mult)
                pred[h] = pr

            def emit_ph2_head(h, copy_fn, pool_fns=None):
                """M1 -> (tmpA, tmpB); fat tiles when pool_fns given."""
                pr = pred[h]
                lhs = [pr[:, xb * 128 : xb * 128 + 128]
                       for xb in range(2)]
                tmpA = upsb.tile([128, XSPLIT], FP16, tag=f"tmpA{h}",
                                 name=f"tmpA{h}")
                tmpB = upsb.tile([128, WP - XSPLIT], FP16, tag=f"tmpB{h}",
                                 name=f"tmpB{h}")

                def tmp_slice(base, wtot):
                    if base < XSPLIT:
                        return tmpA[:, base : base + wtot]
                    return tmpB[:, base - XSPLIT : base - XSPLIT + wtot]

                if pool_fns is None:
                    for c0, nw, halves in XCH_M1:
                        t_ps = outps.tile([128, 512], F32, name="o_ps",
                                          tag="o_ps")
                        for i, xb in enumerate(halves):
                            nc.tensor.matmul(
                                t_ps[:, 0:nw], lhsT=lhs[xb],
                                rhs=wxT[xb][:, c0 : c0 + nw],
                                start=(i == 0), stop=(i == len(halves) - 1),
                            )
                        copy_fn(tmp_slice(c0, nw), t_ps[:, 0:nw])
                    return tmpA, tmpB
                for half, (lo, hi, base) in enumerate(
                        ((0, 2, 0), (2, 5, XSPLIT))):
                    t_ps = pool_fns[half]()
                    for c0, nw, halves in XCH_M1[lo:hi]:
                        for i, xb in enumerate(halves):
                            nc.tensor.matmul(
                                t_ps[:, c0 - base : c0 - base + nw],
                                lhsT=lhs[xb],
                                rhs=wxT[xb][:, c0 : c0 + nw],
                                start=(i == 0),
                                stop=(i == len(halves) - 1),
                            )
                    wtot = (XCH_M1[hi - 1][0] + XCH_M1[hi - 1][1]) - base
                    copy_fn(tmp_slice(base, wtot), t_ps[:, 0:wtot])
                return tmpA, tmpB

            def _tmp_rhs(tmp2, c0, nw):
                tmpA, tmpB = tmp2
                if c0 < XSPLIT:
                    return tmpA[:, c0 : c0 + nw]
                return tmpB[:, c0 - XSPLIT : c0 - XSPLIT + nw]

            dma_tick = [0]

            def emit_ph2_yc(h, tmp2, yc, copy_fn, pool_fns=None,
                            dma_engs=None):
                """pool_fns=None: four 1-bank chunks (mid rows, low PSUM
                pressure).  pool_fns: two 2-bank tiles + fat copies +
                half-row DMAs (tail)."""
                ob = outsb.tile([128, WP], FP16, name="ob", tag="ob")
                if pool_fns is None:
                    for c0, nw in XCH:
                        o_ps = outps.tile([128, 512], F32, name="o_ps",
                                          tag="o_ps")
                        nc.tensor.matmul(
                            o_ps[:, 0:nw],
                            lhsT=wyT[:, yc * 128 : yc * 128 + 128],
                            rhs=_tmp_rhs(tmp2, c0, nw),
                            start=True, stop=True,
                        )
                        copy_fn(ob[:, c0 : c0 + nw], o_ps[:, 0:nw])
                    nc.sync.dma_start(
                        out=out[h, yc * 128 : yc * 128 + 128, :], in_=ob)
                    return
                for half, (cs_, cw) in enumerate(
                        ((0, XSPLIT), (XSPLIT, WP - XSPLIT))):
                    o_ps = pool_fns[half]()
                    for ci in range(2):
                        c0, nw = XCH[half * 2 + ci]
                        nc.tensor.matmul(
                            o_ps[:, c0 - cs_ : c0 - cs_ + nw],
                            lhsT=wyT[:, yc * 128 : yc * 128 + 128],
                            rhs=_tmp_rhs(tmp2, c0, nw),
                            start=True, stop=True,
                        )
                    copy_fn(ob[:, cs_ : cs_ + cw], o_ps[:, 0:cw])
                    if dma_engs is not None:
                        eng = dma_engs[half]
                    else:
                        eng = ENG[DMA_PAT[dma_tick[0] % len(DMA_PAT)]]
                        dma_tick[0] += 1
                    eng.dma_start(
                        out=out[h, yc * 128 : yc * 128 + 128,
                                cs_ : cs_ + cw],
                        in_=ob[:, cs_ : cs_ + cw])

            # sample 0 cost volume + regression; Pool takes POOL_G groups.
            # the last two groups are 2 disparities each so the final
            # [abs -> cs -> exp -> st -> pred] chain is short
            GRP = [(0, 4), (4, 4), (8, 4), (12, 4), (16, 4), (20, 2),
                   (22, 2)]
            NG = len(GRP)
            POOL_G0 = set(cfg["pool_g0"])
            POOL_G1 = set(cfg["pool_g1"])
            for g in range(NG):
                emit_ph1_group(0, *GRP[g],
                               eng=nc.gpsimd if g in POOL_G0 else None,
                               split_exp=(g == NG - 1), last_g=(g == NG - 1))
            emit_ph1_group(1, *GRP[0])
            emit_pred(0)
            tmp0 = emit_ph2_head(0, copy_mid)
            for g in range(1, NG):
                emit_ph1_group(1, *GRP[g],
                               eng=nc.gpsimd if g in POOL_G1 else None,
                               split_exp=(g == NG - 1), last_g=(g == NG - 1))
                emit_ph2_yc(0, tmp0, g - 1, copy_mid)   # rows 0..5
            emit_pred(1)
            ph1_stack.close()  # free cost (4) + s/t (2) banks for the tail
            with tc.tile_pool(name="pstail", bufs=3, space="PSUM") as pstail:
                def tail_tile():
                    return pstail.tile([128, 1024], F32, name="tl",
                                       tag="tl")

                tail_pools = (tail_tile, tail_tile)
                emit_ph2_yc(0, tmp0, 6, copy_tail, tail_pools)
                emit_ph2_yc(0, tmp0, 7, copy_tail, tail_pools)
                tmp1 = emit_ph2_head(1, copy_tail, tail_pools)
                for yc in range(8):
                    emit_ph2_yc(1, tmp1, yc, copy_tail, tail_pools)
    nc.compile()
    return nc


_NC_CACHE = [None]


def kernel(feat_l, feat_r, img_h, img_w):
    feat_l = np.asarray(feat_l, dtype=np.float32)
    feat_r = np.asarray(feat_r, dtype=np.float32)
    assert int(img_h) == HI and int(img_w) == WI
    assert feat_l.shape == (B, C, H0, W0)

    if _NC_CACHE[0] is None:
        _NC_CACHE[0] = _build()
    nc = _NC_CACHE[0]

    consts = _host_consts()
    in_maps = []
    for c in range(NCORES):
        fl = _pack_feat(feat_l[SPC * c : SPC * c + SPC].astype(np.float16))
        fr = _pack_feat_padded(
            feat_r[SPC * c : SPC * c + SPC].astype(np.float16))
        in_maps.append({"lf": fl, "rf": fr, **consts})

    res = run_bass_kernel_spmd(nc, in_maps, core_ids=list(range(NCORES)),
                               trace=_TRACE[0])
    outs = [res.results[i]["out"].astype(np.float32) for i in range(NCORES)]
    full = np.concatenate(outs, axis=0).reshape(B, 1, HI, WI)
    kernel._last_exec_ns = res.exec_time_ns
    return full
